# revision 1
# baseline (speedup 1.0000x reference)
"""Trainium2 Bass kernel for nn_AttentionTypeEnsembleSheafLearner.

Reference computation (per edge e with endpoints (r, c) and type t):
    h   = concat(x[r], x[c])                # [2C] = [256]
    mu, var = mean/var over the 256 features (non-affine LN stats)
    xh  = (h - mu) * rsqrt(var + eps)
    h1  = relu((xh * gamma[t] + beta[t]) @ W1[t] + b1[t])   # [64]
    o   = h1 @ W2[t] + b2[t]                                # [16]
    out = I4 - softmax(o.reshape(4,4), axis=-1)

Strategy (8 NeuronCores, data-parallel over edges):
  * Host folds the per-type affine (gamma/beta) into W1/b1 (exact algebra),
    and precomputes the per-edge LN scalars (inv_std, -mu*inv_std) from
    per-node sum/sumsq — O(E) scalar work, shipped alongside the indices.
  * Edges of each type are dealt round-robin across the 8 cores so every core
    has the same per-type tile counts -> one SPMD program for all cores.
  * Per 128-edge tile (one type per tile): dma_gather of x rows for both
    endpoints (batched, uint16 indices), LN normalize (DVE dual-op
    tensor_scalar), PE transpose, 2-chunk matmul (256->64), ReLU+bias (ACT),
    matmul (64->16), batched softmax, I - attn, DMA out.
  * Host scatters per-core outputs back to original edge order.
"""

import math
import os
import sys

import numpy as np

for _p in ("/opt/trn_rl_repo",):
    if _p not in sys.path:
        sys.path.insert(0, _p)

# Hardcoded problem shape (spec: nn_AttentionTypeEnsembleSheafLearner).
N, C, E, T, H, D = 50000, 128, 320000, 8, 64, 4
DD = D * D
EPS = 1e-5
P = 128
NCORES = 8
M_TILES = 16  # 128-edge tiles per gather batch
# "dma_gather": batched uint16-index gather (crashes on this HW/toolchain).
# "indirect1": per-tile [P,1]-offset indirect DMA (HW-proven).
GATHER_MODE = os.environ.get("GATHER_MODE", "indirect1")

_PROGRAM_CACHE: dict = {}


def _build_program(tile_types, B, M):
    import concourse.bacc as bacc
    import concourse.bass as bass
    import concourse.mybir as mybir
    import concourse.tile as tile
    from concourse.masks import make_identity

    f32 = mybir.dt.float32
    i16 = mybir.dt.int16
    Alu = mybir.AluOpType
    Act = mybir.ActivationFunctionType
    X = mybir.AxisListType.X
    NI = M * P  # indices per gather call

    i32 = mybir.dt.int32
    nc = bacc.Bacc(None, target_bir_lowering=False, debug=False)
    x_d = nc.declare_dram_parameter("x", [N, C], f32, isOutput=False)
    if GATHER_MODE == "indirect1":
        idx_d = nc.declare_dram_parameter("idx", [B, P, 2 * M], i32, isOutput=False)
    else:
        idx_d = nc.declare_dram_parameter("idx", [B, P, 2 * (NI // 16)], i16, isOutput=False)
    scal_d = nc.declare_dram_parameter("scal", [B, P, 2 * M], f32, isOutput=False)
    w1_d = nc.declare_dram_parameter("w1", [P, 2 * T * H], f32, isOutput=False)
    w2_d = nc.declare_dram_parameter("w2", [H, T * DD], f32, isOutput=False)
    b1_d = nc.declare_dram_parameter("b1", [H, T], f32, isOutput=False)
    b2_d = nc.declare_dram_parameter("b2", [P, T * DD], f32, isOutput=False)
    eye_d = nc.declare_dram_parameter("eyeb", [P, M * DD], f32, isOutput=False)
    out_d = nc.declare_dram_parameter("out", [B, P, M * DD], f32, isOutput=True)

    with tile.TileContext(nc) as tc:
        with (
            tc.tile_pool(name="const", bufs=1) as cpool,
            tc.tile_pool(name="work", bufs=3) as wpool,
            tc.tile_pool(name="psum", bufs=2, space="PSUM") as ppool,
            tc.tile_pool(name="psumT", bufs=4, space="PSUM") as ptpool,
        ):
            ident = cpool.tile([P, P], f32)
            make_identity(nc, ident[:])
            w1_sb = cpool.tile([P, 2 * T * H], f32)
            nc.sync.dma_start(out=w1_sb[:], in_=w1_d[:, :])
            w1v = w1_sb[:].rearrange("p (c t h) -> p c t h", c=2, t=T)
            w2_sb = cpool.tile([H, T * DD], f32)
            nc.sync.dma_start(out=w2_sb[:], in_=w2_d[:, :])
            w2v = w2_sb[:].rearrange("p (t k) -> p t k", t=T)
            b1_sb = cpool.tile([H, T], f32)
            nc.sync.dma_start(out=b1_sb[:], in_=b1_d[:, :])
            b2_sb = cpool.tile([P, T * DD], f32)
            nc.sync.dma_start(out=b2_sb[:], in_=b2_d[:, :])
            b2v = b2_sb[:].rearrange("p (t k) -> p t k", t=T)
            eye_sb = cpool.tile([P, M * DD], f32)
            nc.sync.dma_start(out=eye_sb[:], in_=eye_d[:, :])

            for b in range(B):
                if GATHER_MODE == "indirect1":
                    idx_sb = wpool.tile([P, 2 * M], i32, tag="idx")
                else:
                    idx_sb = wpool.tile([P, 2 * (NI // 16)], i16, tag="idx")
                nc.sync.dma_start(out=idx_sb[:], in_=idx_d[b, :, :])
                scal_sb = wpool.tile([P, 2 * M], f32, tag="scal")
                nc.sync.dma_start(out=scal_sb[:], in_=scal_d[b, :, :])
                xr = wpool.tile([P, M, C], f32, tag="xr")
                xc = wpool.tile([P, M, C], f32, tag="xc")
                if GATHER_MODE == "indirect1":
                    for m in range(M):
                        nc.gpsimd.indirect_dma_start(
                            out=xr[:, m, :], out_offset=None, in_=x_d[:, :],
                            in_offset=bass.IndirectOffsetOnAxis(
                                ap=idx_sb[:, m : m + 1], axis=0
                            ),
                        )
                        nc.gpsimd.indirect_dma_start(
                            out=xc[:, m, :], out_offset=None, in_=x_d[:, :],
                            in_offset=bass.IndirectOffsetOnAxis(
                                ap=idx_sb[:, M + m : M + m + 1], axis=0
                            ),
                        )
                else:
                    nc.gpsimd.dma_gather(
                        out_ap=xr[:], in_ap=x_d[:, :], idxs_ap=idx_sb[:, 0 : NI // 16],
                        num_idxs=NI, num_idxs_reg=NI, elem_size=C,
                    )
                    nc.gpsimd.dma_gather(
                        out_ap=xc[:], in_ap=x_d[:, :], idxs_ap=idx_sb[:, NI // 16 : 2 * (NI // 16)],
                        num_idxs=NI, num_idxs_reg=NI, elem_size=C,
                    )

                # phase 1: normalize + transpose + PSUM->SBUF copy, all tiles.
                # Dense back-to-back PE transposes keep the PE p-state warm.
                xnTb = wpool.tile([P, M, 2, C], f32, tag="xnTb")
                for m in range(M):
                    xn = wpool.tile([P, 2, C], f32, tag="xn")
                    # (x * inv_std) + (-mu * inv_std)
                    nc.vector.tensor_scalar(
                        out=xn[:, 0, :], in0=xr[:, m, :],
                        scalar1=scal_sb[:, m : m + 1],
                        scalar2=scal_sb[:, M + m : M + m + 1],
                        op0=Alu.mult, op1=Alu.add,
                    )
                    nc.vector.tensor_scalar(
                        out=xn[:, 1, :], in0=xc[:, m, :],
                        scalar1=scal_sb[:, m : m + 1],
                        scalar2=scal_sb[:, M + m : M + m + 1],
                        op0=Alu.mult, op1=Alu.add,
                    )
                    for c in range(2):
                        pT = ptpool.tile([P, P], f32, tag="pT")
                        nc.tensor.transpose(out=pT[:], in_=xn[:, c, :], identity=ident[:])
                        nc.scalar.activation(
                            out=xnTb[:, m, c, :], in_=pT[:], func=Act.Copy,
                            bias=0.0, scale=1.0,
                        )
                # phase 2: dense matmul chain for all tiles.
                o2sb = wpool.tile([P, M, DD], f32, tag="o2sb")
                for m in range(M):
                    t = tile_types[b * M + m]
                    h1p = ppool.tile([H, P], f32, tag="h1p")
                    for c in range(2):
                        nc.tensor.matmul(
                            out=h1p[:], lhsT=w1v[:, c, t, :], rhs=xnTb[:, m, c, :],
                            start=(c == 0), stop=(c == 1),
                        )
                    h1s = wpool.tile([H, P], f32, tag="h1s")
                    nc.scalar.activation(
                        out=h1s[:], in_=h1p[:], func=Act.Relu,
                        bias=b1_sb[:, t : t + 1], scale=1.0,
                    )
                    o2p = ppool.tile([P, DD], f32, tag="o2p")
                    nc.tensor.matmul(
                        out=o2p[:], lhsT=h1s[:], rhs=w2v[:, t, :], start=True, stop=True
                    )
                    nc.vector.tensor_tensor(
                        out=o2sb[:, m, :], in0=o2p[:], in1=b2v[:, t, :], op=Alu.add
                    )

                # --- batched softmax + (I - attn) over [P, M, 4, 4] ---
                o4 = o2sb[:].rearrange("p m (i j) -> p m i j", i=D)
                mx = wpool.tile([P, M, D], f32, tag="mx")
                nc.vector.tensor_reduce(out=mx[:], in_=o4, axis=X, op=Alu.max)
                sm = wpool.tile([P, M, DD], f32, tag="sm")
                sm4 = sm[:].rearrange("p m (i j) -> p m i j", i=D)
                nc.vector.tensor_tensor(
                    out=sm4, in0=o4,
                    in1=mx[:].unsqueeze(3).to_broadcast([P, M, D, D]),
                    op=Alu.subtract,
                )
                nc.scalar.activation(out=sm[:], in_=sm[:], func=Act.Exp)
                sums = wpool.tile([P, M, D], f32, tag="sums")
                nc.vector.tensor_reduce(out=sums[:], in_=sm4, axis=X, op=Alu.add)
                rec = wpool.tile([P, M, D], f32, tag="rec")
                nc.vector.reciprocal(out=rec[:], in_=sums[:])
                nc.vector.tensor_tensor(
                    out=sm4, in0=sm4,
                    in1=rec[:].unsqueeze(3).to_broadcast([P, M, D, D]),
                    op=Alu.mult,
                )
                outf = wpool.tile([P, M * DD], f32, tag="outf")
                nc.vector.tensor_tensor(
                    out=outf[:], in0=eye_sb[:],
                    in1=sm[:].rearrange("p m k -> p (m k)"),
                    op=Alu.subtract,
                )
                nc.sync.dma_start(out=out_d[b, :, :], in_=outf[:])
    nc.compile()
    return nc


def _wrap_idx(ids, NI):
    """dma_gather index layout: unwrapped[i] -> [i % 16, i // 16], replicated
    across the 8 Q7-core partition stripes -> [128, NI // 16] int16."""
    blk = ids.astype(np.uint16).reshape(NI // 16, 16).T
    return np.tile(blk, (8, 1)).astype(np.int16)


def _prepare(x, edge_index, edge_types, gamma, beta, W1, b1, W2, b2):
    x = np.ascontiguousarray(np.asarray(x, dtype=np.float32))
    ei = np.asarray(edge_index).astype(np.int64)
    et = np.asarray(edge_types).astype(np.int64)
    gamma = np.asarray(gamma, dtype=np.float32)
    beta = np.asarray(beta, dtype=np.float32)
    W1 = np.asarray(W1, dtype=np.float32)
    b1 = np.asarray(b1, dtype=np.float32)
    W2 = np.asarray(W2, dtype=np.float32)
    b2 = np.asarray(b2, dtype=np.float32)

    # fold per-type affine LN params into the first MLP layer (exact algebra)
    W1e = gamma[:, :, None] * W1                      # [T, 2C, H]
    b1e = np.einsum("tc,tch->th", beta, W1) + b1      # [T, H]

    # per-edge LN scalars from per-node partial sums
    s_node = x.sum(axis=1, dtype=np.float64)
    q_node = (x.astype(np.float64) ** 2).sum(axis=1)

    order = np.argsort(et, kind="stable")
    counts = np.bincount(et, minlength=T)
    tiles_t = [int(math.ceil(math.ceil(counts[t] / NCORES) / P)) for t in range(T)]
    NT = sum(tiles_t)
    B = int(math.ceil(NT / M_TILES))
    NTP = B * M_TILES
    NI = M_TILES * P

    tile_types = []
    for t in range(T):
        tile_types += [t] * tiles_t[t]
    tile_types += [T - 1] * (NTP - NT)
    tile_types = tuple(tile_types)

    eids = np.full((NCORES, NTP * P), -1, dtype=np.int64)
    start = np.concatenate([[0], np.cumsum(counts)])
    pos = 0
    for t in range(T):
        arr = order[start[t] : start[t + 1]]
        for k in range(NCORES):
            seg = arr[k::NCORES]
            eids[k, pos : pos + len(seg)] = seg
        pos += tiles_t[t] * P

    row, col = ei[0], ei[1]
    if GATHER_MODE == "indirect1":
        idx_host = np.zeros((NCORES, B, P, 2 * M_TILES), dtype=np.int32)
    else:
        idx_host = np.zeros((NCORES, B, P, 2 * (NI // 16)), dtype=np.int16)
    scal_host = np.zeros((NCORES, B, P, 2 * M_TILES), dtype=np.float32)
    for k in range(NCORES):
        e = eids[k]
        safe = np.maximum(e, 0)
        r = np.where(e >= 0, row[safe], 0)
        c = np.where(e >= 0, col[safe], 0)
        ssum = s_node[r] + s_node[c]
        qsum = q_node[r] + q_node[c]
        mu = ssum / (2 * C)
        var = qsum / (2 * C) - mu * mu
        inv = 1.0 / np.sqrt(var + EPS)
        negms = -mu * inv
        for b in range(B):
            sl = slice(b * NI, (b + 1) * NI)
            if GATHER_MODE == "indirect1":
                idx_host[k, b, :, :M_TILES] = (
                    r[sl].astype(np.int32).reshape(M_TILES, P).T
                )
                idx_host[k, b, :, M_TILES:] = (
                    c[sl].astype(np.int32).reshape(M_TILES, P).T
                )
            else:
                idx_host[k, b, :, : NI // 16] = _wrap_idx(r[sl], NI)
                idx_host[k, b, :, NI // 16 :] = _wrap_idx(c[sl], NI)
            # slot (p, m) <- list position m*128+p
            scal_host[k, b, :, :M_TILES] = (
                inv[sl].astype(np.float32).reshape(M_TILES, P).T
            )
            scal_host[k, b, :, M_TILES:] = (
                negms[sl].astype(np.float32).reshape(M_TILES, P).T
            )

    w1_host = np.ascontiguousarray(
        W1e.reshape(T, 2, P, H).transpose(2, 1, 0, 3).reshape(P, 2 * T * H)
    )
    w2_host = np.ascontiguousarray(W2.transpose(1, 0, 2).reshape(H, T * DD))
    b1_host = np.ascontiguousarray(b1e.T)                      # [H, T]
    b2_host = np.ascontiguousarray(
        np.broadcast_to(b2.reshape(1, T * DD), (P, T * DD))
    )
    eye_host = np.ascontiguousarray(
        np.broadcast_to(
            np.tile(np.eye(D, dtype=np.float32).reshape(DD), M_TILES), (P, M_TILES * DD)
        )
    )
    return dict(
        x=x, idx=idx_host, scal=scal_host, w1=w1_host, w2=w2_host, b1=b1_host,
        b2=b2_host, eye=eye_host, eids=eids, tile_types=tile_types, B=B,
    )


_LAST_RESULTS = {}


def kernel(x, edge_index, edge_types, gamma, beta, W1, b1, W2, b2):
    from concourse.bass_utils import run_bass_kernel_spmd

    prep = _prepare(x, edge_index, edge_types, gamma, beta, W1, b1, W2, b2)
    B, tile_types = prep["B"], prep["tile_types"]

    key = (B, M_TILES, GATHER_MODE, tile_types)
    nc = _PROGRAM_CACHE.get(key)
    if nc is None:
        nc = _build_program(tile_types, B, M_TILES)
        _PROGRAM_CACHE[key] = nc

    in_maps = [
        dict(
            x=prep["x"], idx=prep["idx"][k], scal=prep["scal"][k], w1=prep["w1"],
            w2=prep["w2"], b1=prep["b1"], b2=prep["b2"], eyeb=prep["eye"],
        )
        for k in range(NCORES)
    ]
    trace = bool(int(os.environ.get("KERNEL_TRACE", "0")))
    res = run_bass_kernel_spmd(
        nc, in_maps, core_ids=list(range(NCORES)), trace=trace
    )
    _LAST_RESULTS["res"] = res

    out = np.zeros((E, DD), dtype=np.float32)
    for k in range(NCORES):
        o = (
            res.results[k]["out"]
            .reshape(B, P, M_TILES, DD)
            .transpose(0, 2, 1, 3)
            .reshape(-1, DD)
        )
        e = prep["eids"][k]
        valid = e >= 0
        out[e[valid]] = o[valid]
    return out.reshape(E, D, D)



# revision 4
# speedup vs baseline: 1.1806x; 1.1806x over previous
"""Trainium2 Bass kernel for nn_AttentionTypeEnsembleSheafLearner (v2).

Reference computation (per edge e with endpoints (r, c) and type t):
    h   = concat(x[r], x[c])                # [2C] = [256]
    mu, var = mean/var over the 256 features (non-affine LN stats)
    xh  = (h - mu) * rsqrt(var + eps)
    h1  = relu((xh * gamma[t] + beta[t]) @ W1[t] + b1[t])   # [64]
    o   = h1 @ W2[t] + b2[t]                                # [16]
    out = I4 - softmax(o.reshape(4,4), axis=-1)

v2 strategy (8 NeuronCores, data-parallel over edges):
  * Edges are dealt round-robin across cores, then grouped per core by
    (type, endpoint-range class).  The class splits edges by whether the
    r/c node ids are < 32768 (dma_gather's int16 index limit), so each
    segment can gather its endpoint rows with batched dma_gather calls
    (transpose=True) from the matching half of a bf16 copy of x.  The
    gather lands feature-major ([128 feats, edges]) — no PE transposes.
  * LayerNorm is folded into the matmuls: with std_e = sqrt(var+eps) and
    total[h,e] = (W1e^T h_cat)[h,e] - mu_e*u[h] + std_e*b1e[h]
    (u = column sums of W1e), we have  z = inv_e * total  and since
    inv_e > 0:  relu(z) = inv_e * relu(total).  The -mu/std terms ride a
    K=2 matmul chunk; inv_e is applied after mm2 (where edges sit on
    partitions) as a per-partition scalar multiply.  b2 rides an
    augmented K=65 row of mm2 scaled by std_e so it survives the final
    inv multiply exactly.
  * All matmul operands bf16 (tolerance 2e-2; measured ~1e-3), PSUM f32.
  * Softmax + (I - attn) run as a few mega-batched DVE/Act ops.
"""

import math
import os
import sys

import numpy as np
import ml_dtypes

for _p in ("/opt/trn_rl_repo",):
    if _p not in sys.path:
        sys.path.insert(0, _p)

bf16 = ml_dtypes.bfloat16

# Hardcoded problem shape (spec: nn_AttentionTypeEnsembleSheafLearner).
N, C, E, T, H, D = 50000, 128, 320000, 8, 64, 4
DD = D * D
EPS = 1e-5
P = 128
NCORES = 8
NLO = 32768          # dma_gather int16 index limit
GCHUNK = 768         # max idxs per dma_gather call (HW cap ~1024 crashes)
STRIP = 512          # edges per compute strip (PSUM bank = 512 f32)
SMCH = 4             # softmax mega-chunks

_PROGRAM_CACHE: dict = {}


def _plan_segments(tiles_tc):
    """tiles_tc: [T][4] tile counts. Returns (segments, ntp) where each
    segment is (t, cls, start_tile, n_tiles)."""
    segments = []
    pos = 0
    for t in range(T):
        for cls in range(4):
            n = tiles_tc[t][cls]
            if n:
                segments.append((t, cls, pos, n))
                pos += n
    if pos % 16:
        padt = 16 - pos % 16
        segments.append((7, 0, pos, padt))
        pos += padt
    return segments, pos


def _chunks(total, step):
    out = []
    o = 0
    while o < total:
        out.append((o, min(step, total - o)))
        o += step
    return out


def _build_program(segments, ntp, idxcols):
    import concourse.bacc as bacc
    import concourse.mybir as mybir
    import concourse.tile as tile

    f32 = mybir.dt.float32
    bf = mybir.dt.bfloat16
    i16 = mybir.dt.int16
    Alu = mybir.AluOpType
    Act = mybir.ActivationFunctionType
    X = mybir.AxisListType.X

    NE = ntp * P  # padded edge slots
    segmax = max(n for (_, _, _, n) in segments) * P

    nc = bacc.Bacc(None, target_bir_lowering=False, debug=False)
    x_d = nc.declare_dram_parameter("xbf", [N, C], bf, isOutput=False)
    idx_d = nc.declare_dram_parameter("idxw", [P, idxcols], i16, isOutput=False)
    scal2_d = nc.declare_dram_parameter("scal2", [2, NE], bf, isOutput=False)
    std1_d = nc.declare_dram_parameter("std1", [1, NE], bf, isOutput=False)
    invc_d = nc.declare_dram_parameter("invc", [P, ntp], f32, isOutput=False)
    w1a_d = nc.declare_dram_parameter("w1a", [P, T * H], bf, isOutput=False)
    w1b_d = nc.declare_dram_parameter("w1b", [P, T * H], bf, isOutput=False)
    wc_d = nc.declare_dram_parameter("wc", [2, T * H], bf, isOutput=False)
    w2_d = nc.declare_dram_parameter("w2aug", [H + 1, T * DD], bf, isOutput=False)
    eye_d = nc.declare_dram_parameter("eyeb", [P, (ntp // SMCH) * DD], f32, isOutput=False)
    out_d = nc.declare_dram_parameter("out", [P, ntp * DD], f32, isOutput=True)

    x_lo = x_d[0:NLO, :]
    x_hi = x_d[NLO:N, :]

    with tile.TileContext(nc) as tc:
        with (
            tc.tile_pool(name="const", bufs=1) as cpool,
            tc.tile_pool(name="seg", bufs=2) as spool,
            tc.tile_pool(name="work", bufs=3) as wpool,
            tc.tile_pool(name="sm", bufs=2) as mpool,
            tc.tile_pool(name="psum", bufs=2, space="PSUM") as ppool,
            tc.tile_pool(name="psum2", bufs=4, space="PSUM") as ptpool,
        ):
            idx_sb = cpool.tile([P, idxcols], i16)
            nc.sync.dma_start(out=idx_sb[:], in_=idx_d[:, :])
            w1a_sb = cpool.tile([P, T * H], bf)
            nc.sync.dma_start(out=w1a_sb[:], in_=w1a_d[:, :])
            w1a_v = w1a_sb[:].rearrange("p (t h) -> p t h", t=T)
            w1b_sb = cpool.tile([P, T * H], bf)
            nc.sync.dma_start(out=w1b_sb[:], in_=w1b_d[:, :])
            w1b_v = w1b_sb[:].rearrange("p (t h) -> p t h", t=T)
            wc_sb = cpool.tile([2, T * H], bf)
            nc.sync.dma_start(out=wc_sb[:], in_=wc_d[:, :])
            wc_v = wc_sb[:].rearrange("p (t h) -> p t h", t=T)
            w2_sb = cpool.tile([H + 1, T * DD], bf)
            nc.sync.dma_start(out=w2_sb[:], in_=w2_d[:, :])
            w2_v = w2_sb[:].rearrange("p (t k) -> p t k", t=T)
            invc_sb = cpool.tile([P, ntp], f32)
            nc.sync.dma_start(out=invc_sb[:], in_=invc_d[:, :])
            eye_sb = cpool.tile([P, (ntp // SMCH) * DD], f32)
            nc.sync.dma_start(out=eye_sb[:], in_=eye_d[:, :])
            o2big = cpool.tile([P, ntp * DD], f32)
            o2v = o2big[:].rearrange("p (m k) -> p m k", m=ntp)

            icol = [0]

            def gather(seg_tile, src_ap, ni, off):
                cw = ni // 16
                nc.gpsimd.dma_gather(
                    out_ap=seg_tile[:, off : off + ni].rearrange(
                        "p (a n) -> p a n", a=1
                    ),
                    in_ap=src_ap,
                    idxs_ap=idx_sb[:, icol[0] : icol[0] + cw],
                    num_idxs=ni,
                    num_idxs_reg=ni,
                    elem_size=C,
                    transpose=True,
                )
                icol[0] += cw

            for (t, cls, tile0, ntile) in segments:
                ne = ntile * P
                gbase = tile0 * P
                segr = spool.tile([P, segmax], bf, tag="segr")
                segc = spool.tile([P, segmax], bf, tag="segc")
                rsrc = x_lo if cls in (0, 1) else x_hi
                csrc = x_lo if cls in (0, 2) else x_hi
                for off, ni in _chunks(ne, GCHUNK):
                    gather(segr, rsrc, ni, off)
                for off, ni in _chunks(ne, GCHUNK):
                    gather(segc, csrc, ni, off)

                for soff, S in _chunks(ne, STRIP):
                    g0 = gbase + soff
                    scalc = wpool.tile([2, STRIP], bf, tag="scalc")
                    nc.sync.dma_start(
                        out=scalc[:, 0:S], in_=scal2_d[:, g0 : g0 + S]
                    )
                    ps1 = ppool.tile([H, STRIP], f32, tag="ps1")
                    nc.tensor.matmul(
                        out=ps1[:, 0:S], lhsT=w1a_v[:, t, :],
                        rhs=segr[:, soff : soff + S], start=True, stop=False,
                    )
                    nc.tensor.matmul(
                        out=ps1[:, 0:S], lhsT=w1b_v[:, t, :],
                        rhs=segc[:, soff : soff + S], start=False, stop=False,
                    )
                    nc.tensor.matmul(
                        out=ps1[:, 0:S], lhsT=wc_v[:, t, :], rhs=scalc[:, 0:S],
                        start=False, stop=True,
                    )
                    h1 = wpool.tile([H + 1, STRIP], bf, tag="h1")
                    nc.scalar.activation(
                        out=h1[0:H, 0:S], in_=ps1[:, 0:S], func=Act.Relu
                    )
                    nc.sync.dma_start(
                        out=h1[H : H + 1, 0:S], in_=std1_d[:, g0 : g0 + S]
                    )
                    for i in range(S // P):
                        g = (g0 + i * P) // P
                        ps2 = ptpool.tile([P, DD], f32, tag="ps2")
                        nc.tensor.matmul(
                            out=ps2[:], lhsT=h1[:, i * P : (i + 1) * P],
                            rhs=w2_v[:, t, :], start=True, stop=True,
                        )
                        nc.vector.tensor_scalar(
                            out=o2v[:, g, :], in0=ps2[:],
                            scalar1=invc_sb[:, g : g + 1], scalar2=None,
                            op0=Alu.mult,
                        )

            # --- mega-batched softmax + (I - attn) ---
            mb = ntp // SMCH
            for s in range(SMCH):
                sl = o2v[:, s * mb : (s + 1) * mb, :]
                o4 = sl.rearrange("p m (i j) -> p m i j", i=D)
                mx = mpool.tile([P, mb, D], f32, tag="mx")
                nc.vector.tensor_reduce(out=mx[:], in_=o4, axis=X, op=Alu.max)
                sm = mpool.tile([P, mb * DD], f32, tag="sm")
                sm4 = sm[:].rearrange("p (m i j) -> p m i j", m=mb, i=D)
                nc.vector.tensor_tensor(
                    out=sm4, in0=o4,
                    in1=mx[:].unsqueeze(3).to_broadcast([P, mb, D, D]),
                    op=Alu.subtract,
                )
                nc.scalar.activation(out=sm[:], in_=sm[:], func=Act.Exp)
                sums = mpool.tile([P, mb, D], f32, tag="sums")
                nc.vector.tensor_reduce(out=sums[:], in_=sm4, axis=X, op=Alu.add)
                rec = mpool.tile([P, mb, D], f32, tag="rec")
                nc.vector.reciprocal(out=rec[:], in_=sums[:])
                nc.vector.tensor_tensor(
                    out=sm4, in0=sm4,
                    in1=rec[:].unsqueeze(3).to_broadcast([P, mb, D, D]),
                    op=Alu.mult,
                )
                outf = mpool.tile([P, mb * DD], f32, tag="outf")
                nc.vector.tensor_tensor(
                    out=outf[:], in0=eye_sb[:], in1=sm[:], op=Alu.subtract
                )
                nc.sync.dma_start(
                    out=out_d[:, s * mb * DD : (s + 1) * mb * DD], in_=outf[:]
                )
    nc.compile()
    return nc


def _wrap_idx(ids, ni):
    """dma_gather index layout: unwrapped[i] -> [i % 16, i // 16], replicated
    across the 8 Q7-core partition stripes -> [128, ni // 16] int16."""
    blk = ids.astype(np.uint16).reshape(ni // 16, 16).T
    return np.tile(blk, (8, 1)).astype(np.int16)


def _prepare(x, edge_index, edge_types, gamma, beta, W1, b1, W2, b2):
    x = np.asarray(x, dtype=np.float32)
    ei = np.asarray(edge_index).astype(np.int64)
    et = np.asarray(edge_types).astype(np.int64)
    gamma = np.asarray(gamma, dtype=np.float32)
    beta = np.asarray(beta, dtype=np.float32)
    W1 = np.asarray(W1, dtype=np.float32)
    b1 = np.asarray(b1, dtype=np.float32)
    W2 = np.asarray(W2, dtype=np.float32)
    b2 = np.asarray(b2, dtype=np.float32)

    # fold per-type affine (gamma/beta) into the first MLP layer
    W1e = gamma[:, :, None] * W1                      # [T, 2C, H]
    b1e = np.einsum("tc,tch->th", beta, W1) + b1      # [T, H]
    u = W1e.sum(axis=1)                               # [T, H]

    # per-edge LN stats from per-node partial sums (f64 for accuracy)
    s_node = x.sum(axis=1, dtype=np.float64)
    q_node = (x.astype(np.float64) ** 2).sum(axis=1)

    row, col = ei[0], ei[1]
    cls_all = (row >= NLO).astype(np.int64) * 2 + (col >= NLO).astype(np.int64)

    # per-core edge lists, grouped by (type, class), dealt round-robin
    percore = [[[None] * 4 for _ in range(T)] for _ in range(NCORES)]
    for t in range(T):
        for cl in range(4):
            sel = np.nonzero((et == t) & (cls_all == cl))[0]
            for k in range(NCORES):
                percore[k][t][cl] = sel[k::NCORES]
    tiles_tc = [
        [
            int(math.ceil(max(len(percore[k][t][cl]) for k in range(NCORES)) / P))
            for cl in range(4)
        ]
        for t in range(T)
    ]
    segments, ntp = _plan_segments(tiles_tc)
    NE = ntp * P

    # per-core edge slot assignment (first matching segment per (t, cl);
    # a trailing pad pseudo-segment may duplicate (7, 0) and stays empty)
    eids = np.full((NCORES, NE), -1, dtype=np.int64)
    for k in range(NCORES):
        for t in range(T):
            for cl in range(4):
                seg = next(
                    (s for s in segments if s[0] == t and s[1] == cl), None
                )
                if seg is None:
                    continue
                _, _, tile0, _ = seg
                arr = percore[k][t][cl]
                eids[k, tile0 * P : tile0 * P + len(arr)] = arr

    idxcols = sum(
        2 * sum(ni // 16 for (_, ni) in _chunks(n * P, GCHUNK))
        for (_, _, _, n) in segments
    )

    idx_host = np.zeros((NCORES, P, idxcols), dtype=np.int16)
    scal2_host = np.zeros((NCORES, 2, NE), dtype=bf16)
    std1_host = np.zeros((NCORES, 1, NE), dtype=bf16)
    invc_host = np.ones((NCORES, P, ntp), dtype=np.float32)

    for k in range(NCORES):
        e = eids[k]
        valid = e >= 0
        safe = np.where(valid, e, 0)
        r = np.where(valid, row[safe], 0)
        c = np.where(valid, col[safe], 0)
        ssum = s_node[r] + s_node[c]
        qsum = q_node[r] + q_node[c]
        mu = ssum / (2 * C)
        var = qsum / (2 * C) - mu * mu
        inv = 1.0 / np.sqrt(var + EPS)
        std = np.sqrt(var + EPS)
        mu = np.where(valid, mu, 0.0)
        inv = np.where(valid, inv, 1.0)
        std = np.where(valid, std, 1.0)

        scal2_host[k, 0, :] = (-mu).astype(bf16)
        scal2_host[k, 1, :] = std.astype(bf16)
        std1_host[k, 0, :] = std.astype(bf16)
        invc_host[k] = inv.astype(np.float32).reshape(ntp, P).T

        ic = 0
        for (t, cl, tile0, ntile) in segments:
            ne = ntile * P
            base = tile0 * P
            rr = r[base : base + ne].copy()
            cc = c[base : base + ne].copy()
            rr = rr - (NLO if cl in (2, 3) else 0)
            cc = cc - (NLO if cl in (1, 3) else 0)
            rr = np.maximum(rr, 0)
            cc = np.maximum(cc, 0)
            for off, ni in _chunks(ne, GCHUNK):
                idx_host[k, :, ic : ic + ni // 16] = _wrap_idx(rr[off : off + ni], ni)
                ic += ni // 16
            for off, ni in _chunks(ne, GCHUNK):
                idx_host[k, :, ic : ic + ni // 16] = _wrap_idx(cc[off : off + ni], ni)
                ic += ni // 16
        assert ic == idxcols

    x_bf = np.ascontiguousarray(x.astype(bf16))
    w1a_host = np.ascontiguousarray(
        W1e[:, :C, :].astype(bf16).transpose(1, 0, 2).reshape(P, T * H)
    )
    w1b_host = np.ascontiguousarray(
        W1e[:, C:, :].astype(bf16).transpose(1, 0, 2).reshape(P, T * H)
    )
    wc_host = np.ascontiguousarray(
        np.stack([u, b1e], axis=1).astype(bf16).transpose(1, 0, 2).reshape(2, T * H)
    )
    w2aug = np.concatenate([W2, b2[:, None, :]], axis=1)  # [T, H+1, DD]
    w2_host = np.ascontiguousarray(
        w2aug.astype(bf16).transpose(1, 0, 2).reshape(H + 1, T * DD)
    )
    mbt = ntp // SMCH
    eye_host = np.ascontiguousarray(
        np.broadcast_to(
            np.tile(np.eye(D, dtype=np.float32).reshape(DD), mbt), (P, mbt * DD)
        )
    )
    return dict(
        xbf=x_bf, idx=idx_host, scal2=scal2_host, std1=std1_host, invc=invc_host,
        w1a=w1a_host, w1b=w1b_host, wc=wc_host, w2aug=w2_host, eye=eye_host,
        eids=eids, segments=tuple(segments), ntp=ntp, idxcols=idxcols,
    )


_LAST_RESULTS = {}


def kernel(x, edge_index, edge_types, gamma, beta, W1, b1, W2, b2):
    from concourse.bass_utils import run_bass_kernel_spmd

    prep = _prepare(x, edge_index, edge_types, gamma, beta, W1, b1, W2, b2)
    segments, ntp, idxcols = prep["segments"], prep["ntp"], prep["idxcols"]

    key = (segments, ntp, idxcols)
    nc = _PROGRAM_CACHE.get(key)
    if nc is None:
        nc = _build_program(segments, ntp, idxcols)
        _PROGRAM_CACHE[key] = nc

    in_maps = [
        dict(
            xbf=prep["xbf"], idxw=prep["idx"][k], scal2=prep["scal2"][k],
            std1=prep["std1"][k], invc=prep["invc"][k], w1a=prep["w1a"],
            w1b=prep["w1b"], wc=prep["wc"], w2aug=prep["w2aug"], eyeb=prep["eye"],
        )
        for k in range(NCORES)
    ]
    trace = bool(int(os.environ.get("KERNEL_TRACE", "0")))
    res = run_bass_kernel_spmd(
        nc, in_maps, core_ids=list(range(NCORES)), trace=trace
    )
    _LAST_RESULTS["res"] = res

    out = np.zeros((E, DD), dtype=np.float32)
    for k in range(NCORES):
        o = (
            np.asarray(res.results[k]["out"])
            .reshape(P, ntp, DD)
            .transpose(1, 0, 2)
            .reshape(-1, DD)
        )
        e = prep["eids"][k]
        valid = e >= 0
        out[e[valid]] = o[valid]
    return out.reshape(E, D, D)


# revision 10
# speedup vs baseline: 2.2247x; 1.8844x over previous
"""Trainium2 Bass kernel for nn_AttentionTypeEnsembleSheafLearner (v2).

Reference computation (per edge e with endpoints (r, c) and type t):
    h   = concat(x[r], x[c])                # [2C] = [256]
    mu, var = mean/var over the 256 features (non-affine LN stats)
    xh  = (h - mu) * rsqrt(var + eps)
    h1  = relu((xh * gamma[t] + beta[t]) @ W1[t] + b1[t])   # [64]
    o   = h1 @ W2[t] + b2[t]                                # [16]
    out = I4 - softmax(o.reshape(4,4), axis=-1)

v2 strategy (8 NeuronCores, data-parallel over edges):
  * Edges are dealt round-robin across cores, then grouped per core by
    (type, endpoint-range class).  The class splits edges by whether the
    r/c node ids are < 32768 (dma_gather's int16 index limit), so each
    segment can gather its endpoint rows with batched dma_gather calls
    (transpose=True) from the matching half of a bf16 copy of x.  The
    gather lands feature-major ([128 feats, edges]) — no PE transposes.
  * LayerNorm is folded into the matmuls: with std_e = sqrt(var+eps) and
    total[h,e] = (W1e^T h_cat)[h,e] - mu_e*u[h] + std_e*b1e[h]
    (u = column sums of W1e), we have  z = inv_e * total  and since
    inv_e > 0:  relu(z) = inv_e * relu(total).  The -mu/std terms ride a
    K=2 matmul chunk; inv_e is applied after mm2 (where edges sit on
    partitions) as a per-partition scalar multiply.  b2 rides an
    augmented K=65 row of mm2 scaled by std_e so it survives the final
    inv multiply exactly.
  * All matmul operands bf16 (tolerance 2e-2; measured ~1e-3), PSUM f32.
  * Softmax + (I - attn) run as a few mega-batched DVE/Act ops.
"""

import math
import os
import sys

import numpy as np
import ml_dtypes

for _p in ("/opt/trn_rl_repo",):
    if _p not in sys.path:
        sys.path.insert(0, _p)

bf16 = ml_dtypes.bfloat16

# Hardcoded problem shape (spec: nn_AttentionTypeEnsembleSheafLearner).
N, C, E, T, H, D = 50000, 128, 320000, 8, 64, 4
DD = D * D
EPS = 1e-5
P = 128
NCORES = 8
NLO = 32768          # dma_gather int16 index limit
GCHUNK = 512         # max idxs per dma_gather call (HW cap ~1024 crashes)
NSWQ = 4             # SWDGE queues — descriptor gen parallelizes across them
STRIP = 512          # edges per compute strip (PSUM bank = 512 f32)
SMCH = 4             # softmax mega-chunks

_PROGRAM_CACHE: dict = {}


def _plan_segments(tiles_tc):
    """tiles_tc: [T][4] tile counts. Returns (segments, ntp) where each
    segment is (t, cls, start_tile, n_tiles)."""
    segments = []
    pos = 0
    for t in range(T):
        for cls in range(4):
            n = tiles_tc[t][cls]
            if n:
                segments.append((t, cls, pos, n))
                pos += n
    if pos % 16:
        padt = 16 - pos % 16
        segments.append((7, 0, pos, padt))
        pos += padt
    return segments, pos


def _chunks(total, step):
    out = []
    o = 0
    while o < total:
        out.append((o, min(step, total - o)))
        o += step
    return out


def _build_program(segments, ntp, idxcols):
    import concourse.bacc as bacc
    import concourse.mybir as mybir
    import concourse.tile as tile

    f32 = mybir.dt.float32
    bf = mybir.dt.bfloat16
    i16 = mybir.dt.int16
    Alu = mybir.AluOpType
    Act = mybir.ActivationFunctionType
    X = mybir.AxisListType.X

    NE = ntp * P  # padded edge slots
    segmax = max(n for (_, _, _, n) in segments) * P

    nc = bacc.Bacc(
        None, target_bir_lowering=False, debug=False, num_swdge_queues=NSWQ,
        dynamic_dma_scratch_size=65536,
    )
    x_d = nc.declare_dram_parameter("xbf", [N, C], bf, isOutput=False)
    idx_d = nc.declare_dram_parameter("idxw", [P, idxcols], i16, isOutput=False)
    scal2_d = nc.declare_dram_parameter("scal2", [2, NE], bf, isOutput=False)
    std1_d = nc.declare_dram_parameter("std1", [1, NE], bf, isOutput=False)
    invc_d = nc.declare_dram_parameter("invc", [P, ntp], f32, isOutput=False)
    w1a_d = nc.declare_dram_parameter("w1a", [P, T * H], bf, isOutput=False)
    w1b_d = nc.declare_dram_parameter("w1b", [P, T * H], bf, isOutput=False)
    wc_d = nc.declare_dram_parameter("wc", [2, T * H], bf, isOutput=False)
    w2_d = nc.declare_dram_parameter("w2aug", [H + 1, T * DD], bf, isOutput=False)
    eye_d = nc.declare_dram_parameter("eyeb", [P, (ntp // SMCH) * DD], f32, isOutput=False)
    out_d = nc.declare_dram_parameter("out", [P, ntp * DD], f32, isOutput=True)

    x_lo = x_d[0:NLO, :]
    x_hi = x_d[NLO:N, :]

    with tile.TileContext(nc) as tc:
        with (
            tc.tile_pool(name="const", bufs=1) as cpool,
            tc.tile_pool(name="seg", bufs=2) as spool,
            tc.tile_pool(name="work", bufs=3) as wpool,
            tc.tile_pool(name="sm", bufs=2) as mpool,
            tc.tile_pool(name="psum", bufs=2, space="PSUM") as ppool,
            tc.tile_pool(name="psum2", bufs=4, space="PSUM") as ptpool,
        ):
            idx_sb = cpool.tile([P, idxcols], i16)
            nc.sync.dma_start(out=idx_sb[:], in_=idx_d[:, :])
            w1a_sb = cpool.tile([P, T * H], bf)
            nc.sync.dma_start(out=w1a_sb[:], in_=w1a_d[:, :])
            w1a_v = w1a_sb[:].rearrange("p (t h) -> p t h", t=T)
            w1b_sb = cpool.tile([P, T * H], bf)
            nc.sync.dma_start(out=w1b_sb[:], in_=w1b_d[:, :])
            w1b_v = w1b_sb[:].rearrange("p (t h) -> p t h", t=T)
            wc_sb = cpool.tile([2, T * H], bf)
            nc.sync.dma_start(out=wc_sb[:], in_=wc_d[:, :])
            wc_v = wc_sb[:].rearrange("p (t h) -> p t h", t=T)
            w2_sb = cpool.tile([H + 1, T * DD], bf)
            nc.sync.dma_start(out=w2_sb[:], in_=w2_d[:, :])
            w2_v = w2_sb[:].rearrange("p (t k) -> p t k", t=T)
            invc_sb = cpool.tile([P, ntp], f32)
            nc.sync.dma_start(out=invc_sb[:], in_=invc_d[:, :])
            eye_sb = cpool.tile([P, (ntp // SMCH) * DD], f32)
            nc.sync.dma_start(out=eye_sb[:], in_=eye_d[:, :])
            o2big = cpool.tile([P, ntp * DD], f32)
            o2v = o2big[:].rearrange("p (m k) -> p m k", m=ntp)

            icol = [0, 0]

            def gather(seg_tile, src_ap, ni, off):
                cw = ni // 16
                nc.gpsimd.dma_gather(
                    out_ap=seg_tile[:, off : off + ni].rearrange(
                        "p (a n) -> p a n", a=1
                    ),
                    in_ap=src_ap,
                    idxs_ap=idx_sb[:, icol[0] : icol[0] + cw],
                    num_idxs=ni,
                    num_idxs_reg=ni,
                    elem_size=C,
                    transpose=True,
                    queue_num=icol[1] % NSWQ,
                )
                icol[0] += cw

            for si, (t, cls, tile0, ntile) in enumerate(segments):
                ne = ntile * P
                gbase = tile0 * P
                segr = spool.tile([P, segmax], bf, tag="segr")
                segc = spool.tile([P, segmax], bf, tag="segc")
                rsrc = x_lo if cls in (0, 1) else x_hi
                csrc = x_lo if cls in (0, 2) else x_hi
                icol[1] = 2 * si  # r-gathers on one queue, c-gathers on the next
                for off, ni in _chunks(ne, GCHUNK):
                    gather(segr, rsrc, ni, off)
                icol[1] = 2 * si + 1
                for off, ni in _chunks(ne, GCHUNK):
                    gather(segc, csrc, ni, off)

                for soff, S in _chunks(ne, STRIP):
                    g0 = gbase + soff
                    scalc = wpool.tile([2, STRIP], bf, tag="scalc")
                    nc.sync.dma_start(
                        out=scalc[:, 0:S], in_=scal2_d[:, g0 : g0 + S]
                    )
                    ps1 = ppool.tile([H, STRIP], f32, tag="ps1")
                    nc.tensor.matmul(
                        out=ps1[:, 0:S], lhsT=w1a_v[:, t, :],
                        rhs=segr[:, soff : soff + S], start=True, stop=False,
                    )
                    nc.tensor.matmul(
                        out=ps1[:, 0:S], lhsT=w1b_v[:, t, :],
                        rhs=segc[:, soff : soff + S], start=False, stop=False,
                    )
                    nc.tensor.matmul(
                        out=ps1[:, 0:S], lhsT=wc_v[:, t, :], rhs=scalc[:, 0:S],
                        start=False, stop=True,
                    )
                    h1 = wpool.tile([H + 1, STRIP], bf, tag="h1")
                    nc.scalar.activation(
                        out=h1[0:H, 0:S], in_=ps1[:, 0:S], func=Act.Relu
                    )
                    nc.sync.dma_start(
                        out=h1[H : H + 1, 0:S], in_=std1_d[:, g0 : g0 + S]
                    )
                    for i in range(S // P):
                        g = (g0 + i * P) // P
                        ps2 = ptpool.tile([P, DD], f32, tag="ps2")
                        nc.tensor.matmul(
                            out=ps2[:], lhsT=h1[:, i * P : (i + 1) * P],
                            rhs=w2_v[:, t, :], start=True, stop=True,
                        )
                        nc.vector.tensor_scalar(
                            out=o2v[:, g, :], in0=ps2[:],
                            scalar1=invc_sb[:, g : g + 1], scalar2=None,
                            op0=Alu.mult,
                        )

            # --- mega-batched softmax + (I - attn) ---
            mb = ntp // SMCH
            for s in range(SMCH):
                sl = o2v[:, s * mb : (s + 1) * mb, :]
                o4 = sl.rearrange("p m (i j) -> p m i j", i=D)
                mx = mpool.tile([P, mb, D], f32, tag="mx")
                nc.vector.tensor_reduce(out=mx[:], in_=o4, axis=X, op=Alu.max)
                sm = mpool.tile([P, mb * DD], f32, tag="sm")
                sm4 = sm[:].rearrange("p (m i j) -> p m i j", m=mb, i=D)
                nc.vector.tensor_tensor(
                    out=sm4, in0=o4,
                    in1=mx[:].unsqueeze(3).to_broadcast([P, mb, D, D]),
                    op=Alu.subtract,
                )
                nc.scalar.activation(out=sm[:], in_=sm[:], func=Act.Exp)
                sums = mpool.tile([P, mb, D], f32, tag="sums")
                nc.vector.tensor_reduce(out=sums[:], in_=sm4, axis=X, op=Alu.add)
                rec = mpool.tile([P, mb, D], f32, tag="rec")
                nc.vector.reciprocal(out=rec[:], in_=sums[:])
                nc.vector.tensor_tensor(
                    out=sm4, in0=sm4,
                    in1=rec[:].unsqueeze(3).to_broadcast([P, mb, D, D]),
                    op=Alu.mult,
                )
                outf = mpool.tile([P, mb * DD], f32, tag="outf")
                nc.vector.tensor_tensor(
                    out=outf[:], in0=eye_sb[:], in1=sm[:], op=Alu.subtract
                )
                nc.sync.dma_start(
                    out=out_d[:, s * mb * DD : (s + 1) * mb * DD], in_=outf[:]
                )
    nc.compile()
    return nc


def _wrap_idx(ids, ni):
    """dma_gather index layout: unwrapped[i] -> [i % 16, i // 16], replicated
    across the 8 Q7-core partition stripes -> [128, ni // 16] int16."""
    blk = ids.astype(np.uint16).reshape(ni // 16, 16).T
    return np.tile(blk, (8, 1)).astype(np.int16)


def _prepare(x, edge_index, edge_types, gamma, beta, W1, b1, W2, b2):
    x = np.asarray(x, dtype=np.float32)
    ei = np.asarray(edge_index).astype(np.int64)
    et = np.asarray(edge_types).astype(np.int64)
    gamma = np.asarray(gamma, dtype=np.float32)
    beta = np.asarray(beta, dtype=np.float32)
    W1 = np.asarray(W1, dtype=np.float32)
    b1 = np.asarray(b1, dtype=np.float32)
    W2 = np.asarray(W2, dtype=np.float32)
    b2 = np.asarray(b2, dtype=np.float32)

    # fold per-type affine (gamma/beta) into the first MLP layer
    W1e = gamma[:, :, None] * W1                      # [T, 2C, H]
    b1e = np.einsum("tc,tch->th", beta, W1) + b1      # [T, H]
    u = W1e.sum(axis=1)                               # [T, H]

    # per-edge LN stats from per-node partial sums (f64 for accuracy)
    s_node = x.sum(axis=1, dtype=np.float64)
    q_node = (x.astype(np.float64) ** 2).sum(axis=1)

    row, col = ei[0], ei[1]
    cls_all = (row >= NLO).astype(np.int64) * 2 + (col >= NLO).astype(np.int64)

    # per-core edge lists, grouped by (type, class), dealt round-robin
    percore = [[[None] * 4 for _ in range(T)] for _ in range(NCORES)]
    for t in range(T):
        for cl in range(4):
            sel = np.nonzero((et == t) & (cls_all == cl))[0]
            for k in range(NCORES):
                percore[k][t][cl] = sel[k::NCORES]
    tiles_tc = [
        [
            int(math.ceil(max(len(percore[k][t][cl]) for k in range(NCORES)) / P))
            for cl in range(4)
        ]
        for t in range(T)
    ]
    segments, ntp = _plan_segments(tiles_tc)
    NE = ntp * P

    # per-core edge slot assignment (first matching segment per (t, cl);
    # a trailing pad pseudo-segment may duplicate (7, 0) and stays empty)
    eids = np.full((NCORES, NE), -1, dtype=np.int64)
    for k in range(NCORES):
        for t in range(T):
            for cl in range(4):
                seg = next(
                    (s for s in segments if s[0] == t and s[1] == cl), None
                )
                if seg is None:
                    continue
                _, _, tile0, _ = seg
                arr = percore[k][t][cl]
                eids[k, tile0 * P : tile0 * P + len(arr)] = arr

    idxcols = sum(
        2 * sum(ni // 16 for (_, ni) in _chunks(n * P, GCHUNK))
        for (_, _, _, n) in segments
    )

    idx_host = np.zeros((NCORES, P, idxcols), dtype=np.int16)
    scal2_host = np.zeros((NCORES, 2, NE), dtype=bf16)
    std1_host = np.zeros((NCORES, 1, NE), dtype=bf16)
    invc_host = np.ones((NCORES, P, ntp), dtype=np.float32)

    for k in range(NCORES):
        e = eids[k]
        valid = e >= 0
        safe = np.where(valid, e, 0)
        r = np.where(valid, row[safe], 0)
        c = np.where(valid, col[safe], 0)
        ssum = s_node[r] + s_node[c]
        qsum = q_node[r] + q_node[c]
        mu = ssum / (2 * C)
        var = qsum / (2 * C) - mu * mu
        inv = 1.0 / np.sqrt(var + EPS)
        std = np.sqrt(var + EPS)
        mu = np.where(valid, mu, 0.0)
        inv = np.where(valid, inv, 1.0)
        std = np.where(valid, std, 1.0)

        scal2_host[k, 0, :] = (-mu).astype(bf16)
        scal2_host[k, 1, :] = std.astype(bf16)
        std1_host[k, 0, :] = std.astype(bf16)
        invc_host[k] = inv.astype(np.float32).reshape(ntp, P).T

        ic = 0
        for (t, cl, tile0, ntile) in segments:
            ne = ntile * P
            base = tile0 * P
            rr = r[base : base + ne].copy()
            cc = c[base : base + ne].copy()
            rr = rr - (NLO if cl in (2, 3) else 0)
            cc = cc - (NLO if cl in (1, 3) else 0)
            rr = np.maximum(rr, 0)
            cc = np.maximum(cc, 0)
            for off, ni in _chunks(ne, GCHUNK):
                idx_host[k, :, ic : ic + ni // 16] = _wrap_idx(rr[off : off + ni], ni)
                ic += ni // 16
            for off, ni in _chunks(ne, GCHUNK):
                idx_host[k, :, ic : ic + ni // 16] = _wrap_idx(cc[off : off + ni], ni)
                ic += ni // 16
        assert ic == idxcols

    x_bf = np.ascontiguousarray(x.astype(bf16))
    w1a_host = np.ascontiguousarray(
        W1e[:, :C, :].astype(bf16).transpose(1, 0, 2).reshape(P, T * H)
    )
    w1b_host = np.ascontiguousarray(
        W1e[:, C:, :].astype(bf16).transpose(1, 0, 2).reshape(P, T * H)
    )
    wc_host = np.ascontiguousarray(
        np.stack([u, b1e], axis=1).astype(bf16).transpose(1, 0, 2).reshape(2, T * H)
    )
    w2aug = np.concatenate([W2, b2[:, None, :]], axis=1)  # [T, H+1, DD]
    w2_host = np.ascontiguousarray(
        w2aug.astype(bf16).transpose(1, 0, 2).reshape(H + 1, T * DD)
    )
    mbt = ntp // SMCH
    eye_host = np.ascontiguousarray(
        np.broadcast_to(
            np.tile(np.eye(D, dtype=np.float32).reshape(DD), mbt), (P, mbt * DD)
        )
    )
    return dict(
        xbf=x_bf, idx=idx_host, scal2=scal2_host, std1=std1_host, invc=invc_host,
        w1a=w1a_host, w1b=w1b_host, wc=wc_host, w2aug=w2_host, eye=eye_host,
        eids=eids, segments=tuple(segments), ntp=ntp, idxcols=idxcols,
    )


_LAST_RESULTS = {}


def kernel(x, edge_index, edge_types, gamma, beta, W1, b1, W2, b2):
    from concourse.bass_utils import run_bass_kernel_spmd

    prep = _prepare(x, edge_index, edge_types, gamma, beta, W1, b1, W2, b2)
    segments, ntp, idxcols = prep["segments"], prep["ntp"], prep["idxcols"]

    key = (segments, ntp, idxcols)
    nc = _PROGRAM_CACHE.get(key)
    if nc is None:
        nc = _build_program(segments, ntp, idxcols)
        _PROGRAM_CACHE[key] = nc

    in_maps = [
        dict(
            xbf=prep["xbf"], idxw=prep["idx"][k], scal2=prep["scal2"][k],
            std1=prep["std1"][k], invc=prep["invc"][k], w1a=prep["w1a"],
            w1b=prep["w1b"], wc=prep["wc"], w2aug=prep["w2aug"], eyeb=prep["eye"],
        )
        for k in range(NCORES)
    ]
    trace = bool(int(os.environ.get("KERNEL_TRACE", "0")))
    res = run_bass_kernel_spmd(
        nc, in_maps, core_ids=list(range(NCORES)), trace=trace
    )
    _LAST_RESULTS["res"] = res

    out = np.zeros((E, DD), dtype=np.float32)
    for k in range(NCORES):
        o = (
            np.asarray(res.results[k]["out"])
            .reshape(P, ntp, DD)
            .transpose(1, 0, 2)
            .reshape(-1, DD)
        )
        e = prep["eids"][k]
        valid = e >= 0
        out[e[valid]] = o[valid]
    return out.reshape(E, D, D)


# revision 11
# speedup vs baseline: 2.3570x; 1.0595x over previous
"""Trainium2 Bass kernel for nn_AttentionTypeEnsembleSheafLearner (v2).

Reference computation (per edge e with endpoints (r, c) and type t):
    h   = concat(x[r], x[c])                # [2C] = [256]
    mu, var = mean/var over the 256 features (non-affine LN stats)
    xh  = (h - mu) * rsqrt(var + eps)
    h1  = relu((xh * gamma[t] + beta[t]) @ W1[t] + b1[t])   # [64]
    o   = h1 @ W2[t] + b2[t]                                # [16]
    out = I4 - softmax(o.reshape(4,4), axis=-1)

v2 strategy (8 NeuronCores, data-parallel over edges):
  * Edges are dealt round-robin across cores, then grouped per core by
    (type, endpoint-range class).  The class splits edges by whether the
    r/c node ids are < 32768 (dma_gather's int16 index limit), so each
    segment can gather its endpoint rows with batched dma_gather calls
    (transpose=True) from the matching half of a bf16 copy of x.  The
    gather lands feature-major ([128 feats, edges]) — no PE transposes.
  * LayerNorm is folded into the matmuls: with std_e = sqrt(var+eps) and
    total[h,e] = (W1e^T h_cat)[h,e] - mu_e*u[h] + std_e*b1e[h]
    (u = column sums of W1e), we have  z = inv_e * total  and since
    inv_e > 0:  relu(z) = inv_e * relu(total).  The -mu/std terms ride a
    K=2 matmul chunk; inv_e is applied after mm2 (where edges sit on
    partitions) as a per-partition scalar multiply.  b2 rides an
    augmented K=65 row of mm2 scaled by std_e so it survives the final
    inv multiply exactly.
  * All matmul operands bf16 (tolerance 2e-2; measured ~1e-3), PSUM f32.
  * Softmax + (I - attn) run as a few mega-batched DVE/Act ops.
"""

import math
import os
import sys

import numpy as np
import ml_dtypes

for _p in ("/opt/trn_rl_repo",):
    if _p not in sys.path:
        sys.path.insert(0, _p)

bf16 = ml_dtypes.bfloat16

# Hardcoded problem shape (spec: nn_AttentionTypeEnsembleSheafLearner).
N, C, E, T, H, D = 50000, 128, 320000, 8, 64, 4
DD = D * D
EPS = 1e-5
P = 128
NCORES = 8
NLO = 32768          # dma_gather int16 index limit
GCHUNK = 768         # max idxs per dma_gather call (HW cap ~1024 crashes)
NSWQ = 4             # SWDGE queues — descriptor gen parallelizes across them
STRIP = 512          # edges per compute strip (PSUM bank = 512 f32)
SMCH = 4             # softmax mega-chunks

_PROGRAM_CACHE: dict = {}


def _plan_segments(tiles_tc):
    """tiles_tc: [T][4] tile counts. Returns (segments, ntp) where each
    segment is (t, cls, start_tile, n_tiles)."""
    segments = []
    pos = 0
    for t in range(T):
        for cls in range(4):
            n = tiles_tc[t][cls]
            if n:
                segments.append((t, cls, pos, n))
                pos += n
    if pos % 16:
        padt = 16 - pos % 16
        segments.append((7, 0, pos, padt))
        pos += padt
    return segments, pos


def _chunks(total, step):
    out = []
    o = 0
    while o < total:
        out.append((o, min(step, total - o)))
        o += step
    return out


def _build_program(segments, ntp, idxcols):
    import concourse.bacc as bacc
    import concourse.mybir as mybir
    import concourse.tile as tile

    f32 = mybir.dt.float32
    bf = mybir.dt.bfloat16
    i16 = mybir.dt.int16
    Alu = mybir.AluOpType
    Act = mybir.ActivationFunctionType
    X = mybir.AxisListType.X

    NE = ntp * P  # padded edge slots
    segmax = max(n for (_, _, _, n) in segments) * P

    nc = bacc.Bacc(
        None, target_bir_lowering=False, debug=False, num_swdge_queues=NSWQ,
        dynamic_dma_scratch_size=98304,
    )
    x_d = nc.declare_dram_parameter("xbf", [N, C], bf, isOutput=False)
    idx_d = nc.declare_dram_parameter("idxw", [P, idxcols], i16, isOutput=False)
    scal2_d = nc.declare_dram_parameter("scal2", [2, NE], bf, isOutput=False)
    std1_d = nc.declare_dram_parameter("std1", [1, NE], bf, isOutput=False)
    invc_d = nc.declare_dram_parameter("invc", [P, ntp], f32, isOutput=False)
    w1a_d = nc.declare_dram_parameter("w1a", [P, T * H], bf, isOutput=False)
    w1b_d = nc.declare_dram_parameter("w1b", [P, T * H], bf, isOutput=False)
    wc_d = nc.declare_dram_parameter("wc", [2, T * H], bf, isOutput=False)
    w2_d = nc.declare_dram_parameter("w2aug", [H + 1, T * DD], bf, isOutput=False)
    eye_d = nc.declare_dram_parameter("eyeb", [P, (ntp // SMCH) * DD], f32, isOutput=False)
    out_d = nc.declare_dram_parameter("out", [P, ntp * DD], f32, isOutput=True)

    x_lo = x_d[0:NLO, :]
    x_hi = x_d[NLO:N, :]

    with tile.TileContext(nc) as tc:
        with (
            tc.tile_pool(name="const", bufs=1) as cpool,
            tc.tile_pool(name="seg", bufs=2) as spool,
            tc.tile_pool(name="work", bufs=3) as wpool,
            tc.tile_pool(name="sm", bufs=2) as mpool,
            tc.tile_pool(name="psum", bufs=2, space="PSUM") as ppool,
            tc.tile_pool(name="psum2", bufs=4, space="PSUM") as ptpool,
        ):
            idx_sb = cpool.tile([P, idxcols], i16)
            nc.sync.dma_start(out=idx_sb[:], in_=idx_d[:, :])
            w1a_sb = cpool.tile([P, T * H], bf)
            nc.sync.dma_start(out=w1a_sb[:], in_=w1a_d[:, :])
            w1a_v = w1a_sb[:].rearrange("p (t h) -> p t h", t=T)
            w1b_sb = cpool.tile([P, T * H], bf)
            nc.sync.dma_start(out=w1b_sb[:], in_=w1b_d[:, :])
            w1b_v = w1b_sb[:].rearrange("p (t h) -> p t h", t=T)
            wc_sb = cpool.tile([2, T * H], bf)
            nc.sync.dma_start(out=wc_sb[:], in_=wc_d[:, :])
            wc_v = wc_sb[:].rearrange("p (t h) -> p t h", t=T)
            w2_sb = cpool.tile([H + 1, T * DD], bf)
            nc.sync.dma_start(out=w2_sb[:], in_=w2_d[:, :])
            w2_v = w2_sb[:].rearrange("p (t k) -> p t k", t=T)
            invc_sb = cpool.tile([P, ntp], f32)
            nc.sync.dma_start(out=invc_sb[:], in_=invc_d[:, :])
            eye_sb = cpool.tile([P, (ntp // SMCH) * DD], f32)
            nc.sync.dma_start(out=eye_sb[:], in_=eye_d[:, :])
            o2big = cpool.tile([P, ntp * DD], f32)
            o2v = o2big[:].rearrange("p (m k) -> p m k", m=ntp)

            icol = [0, 0]
            qload = [0] * NSWQ

            def gather(seg_tile, src_ap, ni, off):
                cw = ni // 16
                nc.gpsimd.dma_gather(
                    out_ap=seg_tile[:, off : off + ni].rearrange(
                        "p (a n) -> p a n", a=1
                    ),
                    in_ap=src_ap,
                    idxs_ap=idx_sb[:, icol[0] : icol[0] + cw],
                    num_idxs=ni,
                    num_idxs_reg=ni,
                    elem_size=C,
                    transpose=True,
                    queue_num=icol[1] % NSWQ,
                )
                icol[0] += cw

            for si, (t, cls, tile0, ntile) in enumerate(segments):
                ne = ntile * P
                gbase = tile0 * P
                segr = spool.tile([P, segmax], bf, tag="segr")
                segc = spool.tile([P, segmax], bf, tag="segc")
                rsrc = x_lo if cls in (0, 1) else x_hi
                csrc = x_lo if cls in (0, 2) else x_hi
                icol[1] = min(range(NSWQ), key=lambda q: qload[q])
                qload[icol[1]] += ne
                for off, ni in _chunks(ne, GCHUNK):
                    gather(segr, rsrc, ni, off)
                icol[1] = min(range(NSWQ), key=lambda q: qload[q])
                qload[icol[1]] += ne
                for off, ni in _chunks(ne, GCHUNK):
                    gather(segc, csrc, ni, off)

                for soff, S in _chunks(ne, STRIP):
                    g0 = gbase + soff
                    scalc = wpool.tile([2, STRIP], bf, tag="scalc")
                    nc.sync.dma_start(
                        out=scalc[:, 0:S], in_=scal2_d[:, g0 : g0 + S]
                    )
                    ps1 = ppool.tile([H, STRIP], f32, tag="ps1")
                    nc.tensor.matmul(
                        out=ps1[:, 0:S], lhsT=w1a_v[:, t, :],
                        rhs=segr[:, soff : soff + S], start=True, stop=False,
                    )
                    nc.tensor.matmul(
                        out=ps1[:, 0:S], lhsT=w1b_v[:, t, :],
                        rhs=segc[:, soff : soff + S], start=False, stop=False,
                    )
                    nc.tensor.matmul(
                        out=ps1[:, 0:S], lhsT=wc_v[:, t, :], rhs=scalc[:, 0:S],
                        start=False, stop=True,
                    )
                    h1 = wpool.tile([H + 1, STRIP], bf, tag="h1")
                    nc.scalar.activation(
                        out=h1[0:H, 0:S], in_=ps1[:, 0:S], func=Act.Relu
                    )
                    nc.sync.dma_start(
                        out=h1[H : H + 1, 0:S], in_=std1_d[:, g0 : g0 + S]
                    )
                    for i in range(S // P):
                        g = (g0 + i * P) // P
                        ps2 = ptpool.tile([P, DD], f32, tag="ps2")
                        nc.tensor.matmul(
                            out=ps2[:], lhsT=h1[:, i * P : (i + 1) * P],
                            rhs=w2_v[:, t, :], start=True, stop=True,
                        )
                        nc.vector.tensor_scalar(
                            out=o2v[:, g, :], in0=ps2[:],
                            scalar1=invc_sb[:, g : g + 1], scalar2=None,
                            op0=Alu.mult,
                        )

            # --- mega-batched softmax + (I - attn) ---
            mb = ntp // SMCH
            for s in range(SMCH):
                sl = o2v[:, s * mb : (s + 1) * mb, :]
                o4 = sl.rearrange("p m (i j) -> p m i j", i=D)
                mx = mpool.tile([P, mb, D], f32, tag="mx")
                nc.vector.tensor_reduce(out=mx[:], in_=o4, axis=X, op=Alu.max)
                sm = mpool.tile([P, mb * DD], f32, tag="sm")
                sm4 = sm[:].rearrange("p (m i j) -> p m i j", m=mb, i=D)
                nc.vector.tensor_tensor(
                    out=sm4, in0=o4,
                    in1=mx[:].unsqueeze(3).to_broadcast([P, mb, D, D]),
                    op=Alu.subtract,
                )
                nc.scalar.activation(out=sm[:], in_=sm[:], func=Act.Exp)
                sums = mpool.tile([P, mb, D], f32, tag="sums")
                nc.vector.tensor_reduce(out=sums[:], in_=sm4, axis=X, op=Alu.add)
                rec = mpool.tile([P, mb, D], f32, tag="rec")
                nc.vector.reciprocal(out=rec[:], in_=sums[:])
                nc.vector.tensor_tensor(
                    out=sm4, in0=sm4,
                    in1=rec[:].unsqueeze(3).to_broadcast([P, mb, D, D]),
                    op=Alu.mult,
                )
                outf = mpool.tile([P, mb * DD], f32, tag="outf")
                nc.vector.tensor_tensor(
                    out=outf[:], in0=eye_sb[:], in1=sm[:], op=Alu.subtract
                )
                nc.sync.dma_start(
                    out=out_d[:, s * mb * DD : (s + 1) * mb * DD], in_=outf[:]
                )
    nc.compile()
    return nc


def _wrap_idx(ids, ni):
    """dma_gather index layout: unwrapped[i] -> [i % 16, i // 16], replicated
    across the 8 Q7-core partition stripes -> [128, ni // 16] int16."""
    blk = ids.astype(np.uint16).reshape(ni // 16, 16).T
    return np.tile(blk, (8, 1)).astype(np.int16)


def _prepare(x, edge_index, edge_types, gamma, beta, W1, b1, W2, b2):
    x = np.asarray(x, dtype=np.float32)
    ei = np.asarray(edge_index).astype(np.int64)
    et = np.asarray(edge_types).astype(np.int64)
    gamma = np.asarray(gamma, dtype=np.float32)
    beta = np.asarray(beta, dtype=np.float32)
    W1 = np.asarray(W1, dtype=np.float32)
    b1 = np.asarray(b1, dtype=np.float32)
    W2 = np.asarray(W2, dtype=np.float32)
    b2 = np.asarray(b2, dtype=np.float32)

    # fold per-type affine (gamma/beta) into the first MLP layer
    W1e = gamma[:, :, None] * W1                      # [T, 2C, H]
    b1e = np.einsum("tc,tch->th", beta, W1) + b1      # [T, H]
    u = W1e.sum(axis=1)                               # [T, H]

    # per-edge LN stats from per-node partial sums (f64 for accuracy)
    s_node = x.sum(axis=1, dtype=np.float64)
    q_node = (x.astype(np.float64) ** 2).sum(axis=1)

    row, col = ei[0], ei[1]
    cls_all = (row >= NLO).astype(np.int64) * 2 + (col >= NLO).astype(np.int64)

    # per-core edge lists, grouped by (type, class), dealt round-robin
    percore = [[[None] * 4 for _ in range(T)] for _ in range(NCORES)]
    for t in range(T):
        for cl in range(4):
            sel = np.nonzero((et == t) & (cls_all == cl))[0]
            for k in range(NCORES):
                percore[k][t][cl] = sel[k::NCORES]
    tiles_tc = [
        [
            int(math.ceil(max(len(percore[k][t][cl]) for k in range(NCORES)) / P))
            for cl in range(4)
        ]
        for t in range(T)
    ]
    segments, ntp = _plan_segments(tiles_tc)
    NE = ntp * P

    # per-core edge slot assignment (first matching segment per (t, cl);
    # a trailing pad pseudo-segment may duplicate (7, 0) and stays empty)
    eids = np.full((NCORES, NE), -1, dtype=np.int64)
    for k in range(NCORES):
        for t in range(T):
            for cl in range(4):
                seg = next(
                    (s for s in segments if s[0] == t and s[1] == cl), None
                )
                if seg is None:
                    continue
                _, _, tile0, _ = seg
                arr = percore[k][t][cl]
                eids[k, tile0 * P : tile0 * P + len(arr)] = arr

    idxcols = sum(
        2 * sum(ni // 16 for (_, ni) in _chunks(n * P, GCHUNK))
        for (_, _, _, n) in segments
    )

    idx_host = np.zeros((NCORES, P, idxcols), dtype=np.int16)
    scal2_host = np.zeros((NCORES, 2, NE), dtype=bf16)
    std1_host = np.zeros((NCORES, 1, NE), dtype=bf16)
    invc_host = np.ones((NCORES, P, ntp), dtype=np.float32)

    for k in range(NCORES):
        e = eids[k]
        valid = e >= 0
        safe = np.where(valid, e, 0)
        r = np.where(valid, row[safe], 0)
        c = np.where(valid, col[safe], 0)
        ssum = s_node[r] + s_node[c]
        qsum = q_node[r] + q_node[c]
        mu = ssum / (2 * C)
        var = qsum / (2 * C) - mu * mu
        inv = 1.0 / np.sqrt(var + EPS)
        std = np.sqrt(var + EPS)
        mu = np.where(valid, mu, 0.0)
        inv = np.where(valid, inv, 1.0)
        std = np.where(valid, std, 1.0)

        scal2_host[k, 0, :] = (-mu).astype(bf16)
        scal2_host[k, 1, :] = std.astype(bf16)
        std1_host[k, 0, :] = std.astype(bf16)
        invc_host[k] = inv.astype(np.float32).reshape(ntp, P).T

        ic = 0
        for (t, cl, tile0, ntile) in segments:
            ne = ntile * P
            base = tile0 * P
            rr = r[base : base + ne].copy()
            cc = c[base : base + ne].copy()
            rr = rr - (NLO if cl in (2, 3) else 0)
            cc = cc - (NLO if cl in (1, 3) else 0)
            rr = np.maximum(rr, 0)
            cc = np.maximum(cc, 0)
            for off, ni in _chunks(ne, GCHUNK):
                idx_host[k, :, ic : ic + ni // 16] = _wrap_idx(rr[off : off + ni], ni)
                ic += ni // 16
            for off, ni in _chunks(ne, GCHUNK):
                idx_host[k, :, ic : ic + ni // 16] = _wrap_idx(cc[off : off + ni], ni)
                ic += ni // 16
        assert ic == idxcols

    x_bf = np.ascontiguousarray(x.astype(bf16))
    w1a_host = np.ascontiguousarray(
        W1e[:, :C, :].astype(bf16).transpose(1, 0, 2).reshape(P, T * H)
    )
    w1b_host = np.ascontiguousarray(
        W1e[:, C:, :].astype(bf16).transpose(1, 0, 2).reshape(P, T * H)
    )
    wc_host = np.ascontiguousarray(
        np.stack([u, b1e], axis=1).astype(bf16).transpose(1, 0, 2).reshape(2, T * H)
    )
    w2aug = np.concatenate([W2, b2[:, None, :]], axis=1)  # [T, H+1, DD]
    w2_host = np.ascontiguousarray(
        w2aug.astype(bf16).transpose(1, 0, 2).reshape(H + 1, T * DD)
    )
    mbt = ntp // SMCH
    eye_host = np.ascontiguousarray(
        np.broadcast_to(
            np.tile(np.eye(D, dtype=np.float32).reshape(DD), mbt), (P, mbt * DD)
        )
    )
    return dict(
        xbf=x_bf, idx=idx_host, scal2=scal2_host, std1=std1_host, invc=invc_host,
        w1a=w1a_host, w1b=w1b_host, wc=wc_host, w2aug=w2_host, eye=eye_host,
        eids=eids, segments=tuple(segments), ntp=ntp, idxcols=idxcols,
    )


_LAST_RESULTS = {}


def kernel(x, edge_index, edge_types, gamma, beta, W1, b1, W2, b2):
    from concourse.bass_utils import run_bass_kernel_spmd

    prep = _prepare(x, edge_index, edge_types, gamma, beta, W1, b1, W2, b2)
    segments, ntp, idxcols = prep["segments"], prep["ntp"], prep["idxcols"]

    key = (segments, ntp, idxcols)
    nc = _PROGRAM_CACHE.get(key)
    if nc is None:
        nc = _build_program(segments, ntp, idxcols)
        _PROGRAM_CACHE[key] = nc

    in_maps = [
        dict(
            xbf=prep["xbf"], idxw=prep["idx"][k], scal2=prep["scal2"][k],
            std1=prep["std1"][k], invc=prep["invc"][k], w1a=prep["w1a"],
            w1b=prep["w1b"], wc=prep["wc"], w2aug=prep["w2aug"], eyeb=prep["eye"],
        )
        for k in range(NCORES)
    ]
    trace = bool(int(os.environ.get("KERNEL_TRACE", "0")))
    res = run_bass_kernel_spmd(
        nc, in_maps, core_ids=list(range(NCORES)), trace=trace
    )
    _LAST_RESULTS["res"] = res

    out = np.zeros((E, DD), dtype=np.float32)
    for k in range(NCORES):
        o = (
            np.asarray(res.results[k]["out"])
            .reshape(P, ntp, DD)
            .transpose(1, 0, 2)
            .reshape(-1, DD)
        )
        e = prep["eids"][k]
        valid = e >= 0
        out[e[valid]] = o[valid]
    return out.reshape(E, D, D)


# revision 12
# speedup vs baseline: 2.7886x; 1.1831x over previous
"""Trainium2 Bass kernel for nn_AttentionTypeEnsembleSheafLearner (v2).

Reference computation (per edge e with endpoints (r, c) and type t):
    h   = concat(x[r], x[c])                # [2C] = [256]
    mu, var = mean/var over the 256 features (non-affine LN stats)
    xh  = (h - mu) * rsqrt(var + eps)
    h1  = relu((xh * gamma[t] + beta[t]) @ W1[t] + b1[t])   # [64]
    o   = h1 @ W2[t] + b2[t]                                # [16]
    out = I4 - softmax(o.reshape(4,4), axis=-1)

v2 strategy (8 NeuronCores, data-parallel over edges):
  * Edges are dealt round-robin across cores, then grouped per core by
    (type, endpoint-range class).  The class splits edges by whether the
    r/c node ids are < 32768 (dma_gather's int16 index limit), so each
    segment can gather its endpoint rows with batched dma_gather calls
    (transpose=True) from the matching half of a bf16 copy of x.  The
    gather lands feature-major ([128 feats, edges]) — no PE transposes.
  * LayerNorm is folded into the matmuls: with std_e = sqrt(var+eps) and
    total[h,e] = (W1e^T h_cat)[h,e] - mu_e*u[h] + std_e*b1e[h]
    (u = column sums of W1e), we have  z = inv_e * total  and since
    inv_e > 0:  relu(z) = inv_e * relu(total).  The -mu/std terms ride a
    K=2 matmul chunk; inv_e is applied after mm2 (where edges sit on
    partitions) as a per-partition scalar multiply.  b2 rides an
    augmented K=65 row of mm2 scaled by std_e so it survives the final
    inv multiply exactly.
  * All matmul operands bf16 (tolerance 2e-2; measured ~1e-3), PSUM f32.
  * Softmax + (I - attn) run as a few mega-batched DVE/Act ops.
"""

import math
import os
import sys

import numpy as np
import ml_dtypes

for _p in ("/opt/trn_rl_repo",):
    if _p not in sys.path:
        sys.path.insert(0, _p)

bf16 = ml_dtypes.bfloat16

# Hardcoded problem shape (spec: nn_AttentionTypeEnsembleSheafLearner).
N, C, E, T, H, D = 50000, 128, 320000, 8, 64, 4
DD = D * D
EPS = 1e-5
P = 128
NCORES = 8
NLO = 32768          # dma_gather int16 index limit
GCHUNK = 768         # max idxs per dma_gather call (HW cap ~1024 crashes)
NSWQ = 4             # SWDGE queues — descriptor gen parallelizes across them
STRIP = 512          # edges per compute strip (PSUM bank = 512 f32)
SMCH = 4             # softmax mega-chunks

_PROGRAM_CACHE: dict = {}


def _plan_segments(tiles_tc):
    """tiles_tc: [T][4] tile counts. Returns (segments, ntp) where each
    segment is (t, cls, start_tile, n_tiles)."""
    segments = []
    pos = 0
    for t in range(T):
        for cls in range(4):
            n = tiles_tc[t][cls]
            if n:
                segments.append((t, cls, pos, n))
                pos += n
    if pos % 16:
        padt = 16 - pos % 16
        segments.append((7, 0, pos, padt))
        pos += padt
    return segments, pos


def _chunks(total, step):
    out = []
    o = 0
    while o < total:
        out.append((o, min(step, total - o)))
        o += step
    return out


def _build_program(segments, ntp, idxcols):
    import concourse.bacc as bacc
    import concourse.mybir as mybir
    import concourse.tile as tile

    f32 = mybir.dt.float32
    bf = mybir.dt.bfloat16
    i16 = mybir.dt.int16
    Alu = mybir.AluOpType
    Act = mybir.ActivationFunctionType
    X = mybir.AxisListType.X

    NE = ntp * P  # padded edge slots
    segmax = max(n for (_, _, _, n) in segments) * P

    nc = bacc.Bacc(
        None, target_bir_lowering=False, debug=False, num_swdge_queues=NSWQ,
        dynamic_dma_scratch_size=98304,
    )
    x_d = nc.declare_dram_parameter("xbf", [N, C], bf, isOutput=False)
    idx_d = nc.declare_dram_parameter("idxw", [P, idxcols], i16, isOutput=False)
    scal2_d = nc.declare_dram_parameter("scal2", [2, NE], bf, isOutput=False)
    std1_d = nc.declare_dram_parameter("std1", [1, NE], bf, isOutput=False)
    invc_d = nc.declare_dram_parameter("invc", [P, ntp], f32, isOutput=False)
    w1a_d = nc.declare_dram_parameter("w1a", [P, T * H], bf, isOutput=False)
    w1b_d = nc.declare_dram_parameter("w1b", [P, T * H], bf, isOutput=False)
    wc_d = nc.declare_dram_parameter("wc", [2, T * H], bf, isOutput=False)
    w2_d = nc.declare_dram_parameter("w2aug", [H + 1, T * DD], bf, isOutput=False)
    eye_d = nc.declare_dram_parameter("eyeb", [P, (ntp // SMCH) * DD], f32, isOutput=False)
    out_d = nc.declare_dram_parameter("out", [P, ntp * DD], f32, isOutput=True)

    x_lo = x_d[0:NLO, :]
    x_hi = x_d[NLO:N, :]

    with tile.TileContext(nc) as tc:
        with (
            tc.tile_pool(name="const", bufs=1) as cpool,
            tc.tile_pool(name="seg", bufs=4) as spool,
            tc.tile_pool(name="work", bufs=3) as wpool,
            tc.tile_pool(name="sm", bufs=2) as mpool,
            tc.tile_pool(name="psum", bufs=2, space="PSUM") as ppool,
            tc.tile_pool(name="psum2", bufs=4, space="PSUM") as ptpool,
        ):
            idx_sb = cpool.tile([P, idxcols], i16)
            for c0 in range(0, idxcols, (idxcols + 7) // 8):
                c1 = min(idxcols, c0 + (idxcols + 7) // 8)
                nc.sync.dma_start(out=idx_sb[:, c0:c1], in_=idx_d[:, c0:c1])
            w1a_sb = cpool.tile([P, T * H], bf)
            nc.sync.dma_start(out=w1a_sb[:], in_=w1a_d[:, :])
            w1a_v = w1a_sb[:].rearrange("p (t h) -> p t h", t=T)
            w1b_sb = cpool.tile([P, T * H], bf)
            nc.sync.dma_start(out=w1b_sb[:], in_=w1b_d[:, :])
            w1b_v = w1b_sb[:].rearrange("p (t h) -> p t h", t=T)
            wc_sb = cpool.tile([2, T * H], bf)
            nc.sync.dma_start(out=wc_sb[:], in_=wc_d[:, :])
            wc_v = wc_sb[:].rearrange("p (t h) -> p t h", t=T)
            w2_sb = cpool.tile([H + 1, T * DD], bf)
            nc.sync.dma_start(out=w2_sb[:], in_=w2_d[:, :])
            w2_v = w2_sb[:].rearrange("p (t k) -> p t k", t=T)
            invc_sb = cpool.tile([P, ntp], f32)
            nc.sync.dma_start(out=invc_sb[:], in_=invc_d[:, :])
            eye_sb = cpool.tile([P, (ntp // SMCH) * DD], f32)
            nc.sync.dma_start(out=eye_sb[:], in_=eye_d[:, :])
            o2big = cpool.tile([P, ntp * DD], f32)
            o2v = o2big[:].rearrange("p (m k) -> p m k", m=ntp)

            icol = [0, 0]
            qload = [0] * NSWQ

            def gather(seg_tile, src_ap, ni, off):
                cw = ni // 16
                nc.gpsimd.dma_gather(
                    out_ap=seg_tile[:, off : off + ni].rearrange(
                        "p (a n) -> p a n", a=1
                    ),
                    in_ap=src_ap,
                    idxs_ap=idx_sb[:, icol[0] : icol[0] + cw],
                    num_idxs=ni,
                    num_idxs_reg=ni,
                    elem_size=C,
                    transpose=True,
                    queue_num=icol[1] % NSWQ,
                )
                icol[0] += cw

            for si, (t, cls, tile0, ntile) in enumerate(segments):
                ne = ntile * P
                gbase = tile0 * P
                segr = spool.tile([P, segmax], bf, tag="segr")
                segc = spool.tile([P, segmax], bf, tag="segc")
                rsrc = x_lo if cls in (0, 1) else x_hi
                csrc = x_lo if cls in (0, 2) else x_hi
                icol[1] = min(range(NSWQ), key=lambda q: qload[q])
                qload[icol[1]] += ne
                for off, ni in _chunks(ne, GCHUNK):
                    gather(segr, rsrc, ni, off)
                icol[1] = min(range(NSWQ), key=lambda q: qload[q])
                qload[icol[1]] += ne
                for off, ni in _chunks(ne, GCHUNK):
                    gather(segc, csrc, ni, off)

                for soff, S in _chunks(ne, STRIP):
                    g0 = gbase + soff
                    scalc = wpool.tile([2, STRIP], bf, tag="scalc")
                    nc.sync.dma_start(
                        out=scalc[:, 0:S], in_=scal2_d[:, g0 : g0 + S]
                    )
                    ps1 = ppool.tile([H, STRIP], f32, tag="ps1")
                    nc.tensor.matmul(
                        out=ps1[:, 0:S], lhsT=w1a_v[:, t, :],
                        rhs=segr[:, soff : soff + S], start=True, stop=False,
                    )
                    nc.tensor.matmul(
                        out=ps1[:, 0:S], lhsT=w1b_v[:, t, :],
                        rhs=segc[:, soff : soff + S], start=False, stop=False,
                    )
                    nc.tensor.matmul(
                        out=ps1[:, 0:S], lhsT=wc_v[:, t, :], rhs=scalc[:, 0:S],
                        start=False, stop=True,
                    )
                    h1 = wpool.tile([H + 1, STRIP], bf, tag="h1")
                    nc.scalar.activation(
                        out=h1[0:H, 0:S], in_=ps1[:, 0:S], func=Act.Relu
                    )
                    nc.sync.dma_start(
                        out=h1[H : H + 1, 0:S], in_=std1_d[:, g0 : g0 + S]
                    )
                    for i in range(S // P):
                        g = (g0 + i * P) // P
                        ps2 = ptpool.tile([P, DD], f32, tag="ps2")
                        nc.tensor.matmul(
                            out=ps2[:], lhsT=h1[:, i * P : (i + 1) * P],
                            rhs=w2_v[:, t, :], start=True, stop=True,
                        )
                        nc.vector.tensor_scalar(
                            out=o2v[:, g, :], in0=ps2[:],
                            scalar1=invc_sb[:, g : g + 1], scalar2=None,
                            op0=Alu.mult,
                        )

            # --- mega-batched softmax + (I - attn) ---
            mb = ntp // SMCH
            for s in range(SMCH):
                sl = o2v[:, s * mb : (s + 1) * mb, :]
                o4 = sl.rearrange("p m (i j) -> p m i j", i=D)
                mx = mpool.tile([P, mb, D], f32, tag="mx")
                nc.vector.tensor_reduce(out=mx[:], in_=o4, axis=X, op=Alu.max)
                sm = mpool.tile([P, mb * DD], f32, tag="sm")
                sm4 = sm[:].rearrange("p (m i j) -> p m i j", m=mb, i=D)
                nc.vector.tensor_tensor(
                    out=sm4, in0=o4,
                    in1=mx[:].unsqueeze(3).to_broadcast([P, mb, D, D]),
                    op=Alu.subtract,
                )
                nc.scalar.activation(out=sm[:], in_=sm[:], func=Act.Exp)
                sums = mpool.tile([P, mb, D], f32, tag="sums")
                nc.vector.tensor_reduce(out=sums[:], in_=sm4, axis=X, op=Alu.add)
                rec = mpool.tile([P, mb, D], f32, tag="rec")
                nc.vector.reciprocal(out=rec[:], in_=sums[:])
                nc.vector.tensor_tensor(
                    out=sm4, in0=sm4,
                    in1=rec[:].unsqueeze(3).to_broadcast([P, mb, D, D]),
                    op=Alu.mult,
                )
                outf = mpool.tile([P, mb * DD], f32, tag="outf")
                nc.vector.tensor_tensor(
                    out=outf[:], in0=eye_sb[:], in1=sm[:], op=Alu.subtract
                )
                nc.sync.dma_start(
                    out=out_d[:, s * mb * DD : (s + 1) * mb * DD], in_=outf[:]
                )
    nc.compile()
    return nc


def _wrap_idx(ids, ni):
    """dma_gather index layout: unwrapped[i] -> [i % 16, i // 16], replicated
    across the 8 Q7-core partition stripes -> [128, ni // 16] int16."""
    blk = ids.astype(np.uint16).reshape(ni // 16, 16).T
    return np.tile(blk, (8, 1)).astype(np.int16)


def _prepare(x, edge_index, edge_types, gamma, beta, W1, b1, W2, b2):
    x = np.asarray(x, dtype=np.float32)
    ei = np.asarray(edge_index).astype(np.int64)
    et = np.asarray(edge_types).astype(np.int64)
    gamma = np.asarray(gamma, dtype=np.float32)
    beta = np.asarray(beta, dtype=np.float32)
    W1 = np.asarray(W1, dtype=np.float32)
    b1 = np.asarray(b1, dtype=np.float32)
    W2 = np.asarray(W2, dtype=np.float32)
    b2 = np.asarray(b2, dtype=np.float32)

    # fold per-type affine (gamma/beta) into the first MLP layer
    W1e = gamma[:, :, None] * W1                      # [T, 2C, H]
    b1e = np.einsum("tc,tch->th", beta, W1) + b1      # [T, H]
    u = W1e.sum(axis=1)                               # [T, H]

    # per-edge LN stats from per-node partial sums (f64 for accuracy)
    s_node = x.sum(axis=1, dtype=np.float64)
    q_node = (x.astype(np.float64) ** 2).sum(axis=1)

    row, col = ei[0], ei[1]
    cls_all = (row >= NLO).astype(np.int64) * 2 + (col >= NLO).astype(np.int64)

    # per-core edge lists, grouped by (type, class), dealt round-robin
    percore = [[[None] * 4 for _ in range(T)] for _ in range(NCORES)]
    for t in range(T):
        for cl in range(4):
            sel = np.nonzero((et == t) & (cls_all == cl))[0]
            for k in range(NCORES):
                percore[k][t][cl] = sel[k::NCORES]
    tiles_tc = [
        [
            int(math.ceil(max(len(percore[k][t][cl]) for k in range(NCORES)) / P))
            for cl in range(4)
        ]
        for t in range(T)
    ]
    segments, ntp = _plan_segments(tiles_tc)
    NE = ntp * P

    # per-core edge slot assignment (first matching segment per (t, cl);
    # a trailing pad pseudo-segment may duplicate (7, 0) and stays empty)
    eids = np.full((NCORES, NE), -1, dtype=np.int64)
    for k in range(NCORES):
        for t in range(T):
            for cl in range(4):
                seg = next(
                    (s for s in segments if s[0] == t and s[1] == cl), None
                )
                if seg is None:
                    continue
                _, _, tile0, _ = seg
                arr = percore[k][t][cl]
                eids[k, tile0 * P : tile0 * P + len(arr)] = arr

    idxcols = sum(
        2 * sum(ni // 16 for (_, ni) in _chunks(n * P, GCHUNK))
        for (_, _, _, n) in segments
    )

    idx_host = np.zeros((NCORES, P, idxcols), dtype=np.int16)
    scal2_host = np.zeros((NCORES, 2, NE), dtype=bf16)
    std1_host = np.zeros((NCORES, 1, NE), dtype=bf16)
    invc_host = np.ones((NCORES, P, ntp), dtype=np.float32)

    for k in range(NCORES):
        e = eids[k]
        valid = e >= 0
        safe = np.where(valid, e, 0)
        r = np.where(valid, row[safe], 0)
        c = np.where(valid, col[safe], 0)
        ssum = s_node[r] + s_node[c]
        qsum = q_node[r] + q_node[c]
        mu = ssum / (2 * C)
        var = qsum / (2 * C) - mu * mu
        inv = 1.0 / np.sqrt(var + EPS)
        std = np.sqrt(var + EPS)
        mu = np.where(valid, mu, 0.0)
        inv = np.where(valid, inv, 1.0)
        std = np.where(valid, std, 1.0)

        scal2_host[k, 0, :] = (-mu).astype(bf16)
        scal2_host[k, 1, :] = std.astype(bf16)
        std1_host[k, 0, :] = std.astype(bf16)
        invc_host[k] = inv.astype(np.float32).reshape(ntp, P).T

        ic = 0
        for (t, cl, tile0, ntile) in segments:
            ne = ntile * P
            base = tile0 * P
            rr = r[base : base + ne].copy()
            cc = c[base : base + ne].copy()
            rr = rr - (NLO if cl in (2, 3) else 0)
            cc = cc - (NLO if cl in (1, 3) else 0)
            rr = np.maximum(rr, 0)
            cc = np.maximum(cc, 0)
            for off, ni in _chunks(ne, GCHUNK):
                idx_host[k, :, ic : ic + ni // 16] = _wrap_idx(rr[off : off + ni], ni)
                ic += ni // 16
            for off, ni in _chunks(ne, GCHUNK):
                idx_host[k, :, ic : ic + ni // 16] = _wrap_idx(cc[off : off + ni], ni)
                ic += ni // 16
        assert ic == idxcols

    x_bf = np.ascontiguousarray(x.astype(bf16))
    w1a_host = np.ascontiguousarray(
        W1e[:, :C, :].astype(bf16).transpose(1, 0, 2).reshape(P, T * H)
    )
    w1b_host = np.ascontiguousarray(
        W1e[:, C:, :].astype(bf16).transpose(1, 0, 2).reshape(P, T * H)
    )
    wc_host = np.ascontiguousarray(
        np.stack([u, b1e], axis=1).astype(bf16).transpose(1, 0, 2).reshape(2, T * H)
    )
    w2aug = np.concatenate([W2, b2[:, None, :]], axis=1)  # [T, H+1, DD]
    w2_host = np.ascontiguousarray(
        w2aug.astype(bf16).transpose(1, 0, 2).reshape(H + 1, T * DD)
    )
    mbt = ntp // SMCH
    eye_host = np.ascontiguousarray(
        np.broadcast_to(
            np.tile(np.eye(D, dtype=np.float32).reshape(DD), mbt), (P, mbt * DD)
        )
    )
    return dict(
        xbf=x_bf, idx=idx_host, scal2=scal2_host, std1=std1_host, invc=invc_host,
        w1a=w1a_host, w1b=w1b_host, wc=wc_host, w2aug=w2_host, eye=eye_host,
        eids=eids, segments=tuple(segments), ntp=ntp, idxcols=idxcols,
    )


_LAST_RESULTS = {}


def kernel(x, edge_index, edge_types, gamma, beta, W1, b1, W2, b2):
    from concourse.bass_utils import run_bass_kernel_spmd

    prep = _prepare(x, edge_index, edge_types, gamma, beta, W1, b1, W2, b2)
    segments, ntp, idxcols = prep["segments"], prep["ntp"], prep["idxcols"]

    key = (segments, ntp, idxcols)
    nc = _PROGRAM_CACHE.get(key)
    if nc is None:
        nc = _build_program(segments, ntp, idxcols)
        _PROGRAM_CACHE[key] = nc

    in_maps = [
        dict(
            xbf=prep["xbf"], idxw=prep["idx"][k], scal2=prep["scal2"][k],
            std1=prep["std1"][k], invc=prep["invc"][k], w1a=prep["w1a"],
            w1b=prep["w1b"], wc=prep["wc"], w2aug=prep["w2aug"], eyeb=prep["eye"],
        )
        for k in range(NCORES)
    ]
    trace = bool(int(os.environ.get("KERNEL_TRACE", "0")))
    res = run_bass_kernel_spmd(
        nc, in_maps, core_ids=list(range(NCORES)), trace=trace
    )
    _LAST_RESULTS["res"] = res

    out = np.zeros((E, DD), dtype=np.float32)
    for k in range(NCORES):
        o = (
            np.asarray(res.results[k]["out"])
            .reshape(P, ntp, DD)
            .transpose(1, 0, 2)
            .reshape(-1, DD)
        )
        e = prep["eids"][k]
        valid = e >= 0
        out[e[valid]] = o[valid]
    return out.reshape(E, D, D)


# revision 13
# speedup vs baseline: 2.8875x; 1.0354x over previous
"""Trainium2 Bass kernel for nn_AttentionTypeEnsembleSheafLearner (v2).

Reference computation (per edge e with endpoints (r, c) and type t):
    h   = concat(x[r], x[c])                # [2C] = [256]
    mu, var = mean/var over the 256 features (non-affine LN stats)
    xh  = (h - mu) * rsqrt(var + eps)
    h1  = relu((xh * gamma[t] + beta[t]) @ W1[t] + b1[t])   # [64]
    o   = h1 @ W2[t] + b2[t]                                # [16]
    out = I4 - softmax(o.reshape(4,4), axis=-1)

v2 strategy (8 NeuronCores, data-parallel over edges):
  * Edges are dealt round-robin across cores, then grouped per core by
    (type, endpoint-range class).  The class splits edges by whether the
    r/c node ids are < 32768 (dma_gather's int16 index limit), so each
    segment can gather its endpoint rows with batched dma_gather calls
    (transpose=True) from the matching half of a bf16 copy of x.  The
    gather lands feature-major ([128 feats, edges]) — no PE transposes.
  * LayerNorm is folded into the matmuls: with std_e = sqrt(var+eps) and
    total[h,e] = (W1e^T h_cat)[h,e] - mu_e*u[h] + std_e*b1e[h]
    (u = column sums of W1e), we have  z = inv_e * total  and since
    inv_e > 0:  relu(z) = inv_e * relu(total).  The -mu/std terms ride a
    K=2 matmul chunk; inv_e is applied after mm2 (where edges sit on
    partitions) as a per-partition scalar multiply.  b2 rides an
    augmented K=65 row of mm2 scaled by std_e so it survives the final
    inv multiply exactly.
  * All matmul operands bf16 (tolerance 2e-2; measured ~1e-3), PSUM f32.
  * Softmax + (I - attn) run as a few mega-batched DVE/Act ops.
"""

import math
import os
import sys

import numpy as np
import ml_dtypes

for _p in ("/opt/trn_rl_repo",):
    if _p not in sys.path:
        sys.path.insert(0, _p)

bf16 = ml_dtypes.bfloat16

# Hardcoded problem shape (spec: nn_AttentionTypeEnsembleSheafLearner).
N, C, E, T, H, D = 50000, 128, 320000, 8, 64, 4
DD = D * D
EPS = 1e-5
P = 128
NCORES = 8
NLO = 32768          # dma_gather int16 index limit
GCHUNK = 768         # max idxs per dma_gather call (HW cap ~1024 crashes)
NSWQ = 4             # SWDGE queues — descriptor gen parallelizes across them
STRIP = 512          # edges per compute strip (PSUM bank = 512 f32)
SMCH = 8             # softmax mega-chunks

_PROGRAM_CACHE: dict = {}


def _plan_segments(tiles_tc):
    """tiles_tc: [T][4] tile counts. Returns (segments, ntp) where each
    segment is (t, cls, start_tile, n_tiles)."""
    segments = []
    pos = 0
    for t in range(T):
        for cls in range(4):
            n = tiles_tc[t][cls]
            if n:
                segments.append((t, cls, pos, n))
                pos += n
    if pos % 16:
        padt = 16 - pos % 16
        segments.append((7, 0, pos, padt))
        pos += padt
    return segments, pos


def _chunks(total, step):
    out = []
    o = 0
    while o < total:
        out.append((o, min(step, total - o)))
        o += step
    return out


def _build_program(segments, ntp, idxcols):
    import concourse.bacc as bacc
    import concourse.mybir as mybir
    import concourse.tile as tile

    f32 = mybir.dt.float32
    bf = mybir.dt.bfloat16
    i16 = mybir.dt.int16
    Alu = mybir.AluOpType
    Act = mybir.ActivationFunctionType
    X = mybir.AxisListType.X

    NE = ntp * P  # padded edge slots
    segmax = max(n for (_, _, _, n) in segments) * P

    nc = bacc.Bacc(
        None, target_bir_lowering=False, debug=False, num_swdge_queues=NSWQ,
        dynamic_dma_scratch_size=114688,
    )
    x_d = nc.declare_dram_parameter("xbf", [N, C], bf, isOutput=False)
    idx_d = nc.declare_dram_parameter("idxw", [P, idxcols], i16, isOutput=False)
    scal2_d = nc.declare_dram_parameter("scal2", [2, NE], bf, isOutput=False)
    std1_d = nc.declare_dram_parameter("std1", [1, NE], bf, isOutput=False)
    invc_d = nc.declare_dram_parameter("invc", [P, ntp], f32, isOutput=False)
    w1a_d = nc.declare_dram_parameter("w1a", [P, T * H], bf, isOutput=False)
    w1b_d = nc.declare_dram_parameter("w1b", [P, T * H], bf, isOutput=False)
    wc_d = nc.declare_dram_parameter("wc", [2, T * H], bf, isOutput=False)
    w2_d = nc.declare_dram_parameter("w2aug", [H + 1, T * DD], bf, isOutput=False)
    eye_d = nc.declare_dram_parameter("eyeb", [P, (ntp // SMCH) * DD], f32, isOutput=False)
    out_d = nc.declare_dram_parameter("out", [P, ntp * DD], f32, isOutput=True)

    x_lo = x_d[0:NLO, :]
    x_hi = x_d[NLO:N, :]

    with tile.TileContext(nc) as tc:
        with (
            tc.tile_pool(name="const", bufs=1) as cpool,
            tc.tile_pool(name="seg", bufs=4) as spool,
            tc.tile_pool(name="work", bufs=3) as wpool,
            tc.tile_pool(name="sm", bufs=2) as mpool,
            tc.tile_pool(name="psum", bufs=2, space="PSUM") as ppool,
            tc.tile_pool(name="psum2", bufs=4, space="PSUM") as ptpool,
        ):
            idx_sb = cpool.tile([P, idxcols], i16)
            for c0 in range(0, idxcols, (idxcols + 7) // 8):
                c1 = min(idxcols, c0 + (idxcols + 7) // 8)
                nc.sync.dma_start(out=idx_sb[:, c0:c1], in_=idx_d[:, c0:c1])
            w1a_sb = cpool.tile([P, T * H], bf)
            nc.sync.dma_start(out=w1a_sb[:], in_=w1a_d[:, :])
            w1a_v = w1a_sb[:].rearrange("p (t h) -> p t h", t=T)
            w1b_sb = cpool.tile([P, T * H], bf)
            nc.sync.dma_start(out=w1b_sb[:], in_=w1b_d[:, :])
            w1b_v = w1b_sb[:].rearrange("p (t h) -> p t h", t=T)
            wc_sb = cpool.tile([2, T * H], bf)
            nc.sync.dma_start(out=wc_sb[:], in_=wc_d[:, :])
            wc_v = wc_sb[:].rearrange("p (t h) -> p t h", t=T)
            w2_sb = cpool.tile([H + 1, T * DD], bf)
            nc.sync.dma_start(out=w2_sb[:], in_=w2_d[:, :])
            w2_v = w2_sb[:].rearrange("p (t k) -> p t k", t=T)
            invc_sb = cpool.tile([P, ntp], f32)
            nc.sync.dma_start(out=invc_sb[:], in_=invc_d[:, :])
            eye_sb = cpool.tile([P, (ntp // SMCH) * DD], f32)
            nc.sync.dma_start(out=eye_sb[:], in_=eye_d[:, :])
            o2big = cpool.tile([P, ntp * DD], f32)
            o2v = o2big[:].rearrange("p (m k) -> p m k", m=ntp)

            icol = [0, 0]
            qload = [0] * NSWQ

            def gather(seg_tile, src_ap, ni, off):
                cw = ni // 16
                nc.gpsimd.dma_gather(
                    out_ap=seg_tile[:, off : off + ni].rearrange(
                        "p (a n) -> p a n", a=1
                    ),
                    in_ap=src_ap,
                    idxs_ap=idx_sb[:, icol[0] : icol[0] + cw],
                    num_idxs=ni,
                    num_idxs_reg=ni,
                    elem_size=C,
                    transpose=True,
                    queue_num=icol[1] % NSWQ,
                )
                icol[0] += cw

            for si, (t, cls, tile0, ntile) in enumerate(segments):
                ne = ntile * P
                gbase = tile0 * P
                segr = spool.tile([P, segmax], bf, tag="segr")
                segc = spool.tile([P, segmax], bf, tag="segc")
                rsrc = x_lo if cls in (0, 1) else x_hi
                csrc = x_lo if cls in (0, 2) else x_hi
                icol[1] = min(range(NSWQ), key=lambda q: qload[q])
                qload[icol[1]] += ne
                for off, ni in _chunks(ne, GCHUNK):
                    gather(segr, rsrc, ni, off)
                icol[1] = min(range(NSWQ), key=lambda q: qload[q])
                qload[icol[1]] += ne
                for off, ni in _chunks(ne, GCHUNK):
                    gather(segc, csrc, ni, off)

                for soff, S in _chunks(ne, STRIP):
                    g0 = gbase + soff
                    scalc = wpool.tile([2, STRIP], bf, tag="scalc")
                    nc.sync.dma_start(
                        out=scalc[:, 0:S], in_=scal2_d[:, g0 : g0 + S]
                    )
                    ps1 = ppool.tile([H, STRIP], f32, tag="ps1")
                    nc.tensor.matmul(
                        out=ps1[:, 0:S], lhsT=w1a_v[:, t, :],
                        rhs=segr[:, soff : soff + S], start=True, stop=False,
                    )
                    nc.tensor.matmul(
                        out=ps1[:, 0:S], lhsT=w1b_v[:, t, :],
                        rhs=segc[:, soff : soff + S], start=False, stop=False,
                    )
                    nc.tensor.matmul(
                        out=ps1[:, 0:S], lhsT=wc_v[:, t, :], rhs=scalc[:, 0:S],
                        start=False, stop=True,
                    )
                    h1 = wpool.tile([H + 1, STRIP], bf, tag="h1")
                    nc.scalar.activation(
                        out=h1[0:H, 0:S], in_=ps1[:, 0:S], func=Act.Relu
                    )
                    nc.sync.dma_start(
                        out=h1[H : H + 1, 0:S], in_=std1_d[:, g0 : g0 + S]
                    )
                    for i in range(S // P):
                        g = (g0 + i * P) // P
                        ps2 = ptpool.tile([P, DD], f32, tag="ps2")
                        nc.tensor.matmul(
                            out=ps2[:], lhsT=h1[:, i * P : (i + 1) * P],
                            rhs=w2_v[:, t, :], start=True, stop=True,
                        )
                        nc.vector.tensor_scalar(
                            out=o2v[:, g, :], in0=ps2[:],
                            scalar1=invc_sb[:, g : g + 1], scalar2=None,
                            op0=Alu.mult,
                        )

            # --- mega-batched softmax + (I - attn) ---
            mb = ntp // SMCH
            for s in range(SMCH):
                sl = o2v[:, s * mb : (s + 1) * mb, :]
                o4 = sl.rearrange("p m (i j) -> p m i j", i=D)
                mx = mpool.tile([P, mb, D], f32, tag="mx")
                nc.vector.tensor_reduce(out=mx[:], in_=o4, axis=X, op=Alu.max)
                sm = mpool.tile([P, mb * DD], f32, tag="sm")
                sm4 = sm[:].rearrange("p (m i j) -> p m i j", m=mb, i=D)
                nc.vector.tensor_tensor(
                    out=sm4, in0=o4,
                    in1=mx[:].unsqueeze(3).to_broadcast([P, mb, D, D]),
                    op=Alu.subtract,
                )
                nc.scalar.activation(out=sm[:], in_=sm[:], func=Act.Exp)
                sums = mpool.tile([P, mb, D], f32, tag="sums")
                nc.vector.tensor_reduce(out=sums[:], in_=sm4, axis=X, op=Alu.add)
                rec = mpool.tile([P, mb, D], f32, tag="rec")
                nc.vector.reciprocal(out=rec[:], in_=sums[:])
                nc.vector.tensor_tensor(
                    out=sm4, in0=sm4,
                    in1=rec[:].unsqueeze(3).to_broadcast([P, mb, D, D]),
                    op=Alu.mult,
                )
                outf = mpool.tile([P, mb * DD], f32, tag="outf")
                nc.vector.tensor_tensor(
                    out=outf[:], in0=eye_sb[:], in1=sm[:], op=Alu.subtract
                )
                nc.sync.dma_start(
                    out=out_d[:, s * mb * DD : (s + 1) * mb * DD], in_=outf[:]
                )
    nc.compile()
    return nc


def _wrap_idx(ids, ni):
    """dma_gather index layout: unwrapped[i] -> [i % 16, i // 16], replicated
    across the 8 Q7-core partition stripes -> [128, ni // 16] int16."""
    blk = ids.astype(np.uint16).reshape(ni // 16, 16).T
    return np.tile(blk, (8, 1)).astype(np.int16)


def _prepare(x, edge_index, edge_types, gamma, beta, W1, b1, W2, b2):
    x = np.asarray(x, dtype=np.float32)
    ei = np.asarray(edge_index).astype(np.int64)
    et = np.asarray(edge_types).astype(np.int64)
    gamma = np.asarray(gamma, dtype=np.float32)
    beta = np.asarray(beta, dtype=np.float32)
    W1 = np.asarray(W1, dtype=np.float32)
    b1 = np.asarray(b1, dtype=np.float32)
    W2 = np.asarray(W2, dtype=np.float32)
    b2 = np.asarray(b2, dtype=np.float32)

    # fold per-type affine (gamma/beta) into the first MLP layer
    W1e = gamma[:, :, None] * W1                      # [T, 2C, H]
    b1e = np.einsum("tc,tch->th", beta, W1) + b1      # [T, H]
    u = W1e.sum(axis=1)                               # [T, H]

    # per-edge LN stats from per-node partial sums (f64 for accuracy)
    s_node = x.sum(axis=1, dtype=np.float64)
    q_node = (x.astype(np.float64) ** 2).sum(axis=1)

    row, col = ei[0], ei[1]
    cls_all = (row >= NLO).astype(np.int64) * 2 + (col >= NLO).astype(np.int64)

    # per-core edge lists, grouped by (type, class), dealt round-robin
    percore = [[[None] * 4 for _ in range(T)] for _ in range(NCORES)]
    for t in range(T):
        for cl in range(4):
            sel = np.nonzero((et == t) & (cls_all == cl))[0]
            for k in range(NCORES):
                percore[k][t][cl] = sel[k::NCORES]
    tiles_tc = [
        [
            int(math.ceil(max(len(percore[k][t][cl]) for k in range(NCORES)) / P))
            for cl in range(4)
        ]
        for t in range(T)
    ]
    segments, ntp = _plan_segments(tiles_tc)
    NE = ntp * P

    # per-core edge slot assignment (first matching segment per (t, cl);
    # a trailing pad pseudo-segment may duplicate (7, 0) and stays empty)
    eids = np.full((NCORES, NE), -1, dtype=np.int64)
    for k in range(NCORES):
        for t in range(T):
            for cl in range(4):
                seg = next(
                    (s for s in segments if s[0] == t and s[1] == cl), None
                )
                if seg is None:
                    continue
                _, _, tile0, _ = seg
                arr = percore[k][t][cl]
                eids[k, tile0 * P : tile0 * P + len(arr)] = arr

    idxcols = sum(
        2 * sum(ni // 16 for (_, ni) in _chunks(n * P, GCHUNK))
        for (_, _, _, n) in segments
    )

    idx_host = np.zeros((NCORES, P, idxcols), dtype=np.int16)
    scal2_host = np.zeros((NCORES, 2, NE), dtype=bf16)
    std1_host = np.zeros((NCORES, 1, NE), dtype=bf16)
    invc_host = np.ones((NCORES, P, ntp), dtype=np.float32)

    for k in range(NCORES):
        e = eids[k]
        valid = e >= 0
        safe = np.where(valid, e, 0)
        r = np.where(valid, row[safe], 0)
        c = np.where(valid, col[safe], 0)
        ssum = s_node[r] + s_node[c]
        qsum = q_node[r] + q_node[c]
        mu = ssum / (2 * C)
        var = qsum / (2 * C) - mu * mu
        inv = 1.0 / np.sqrt(var + EPS)
        std = np.sqrt(var + EPS)
        mu = np.where(valid, mu, 0.0)
        inv = np.where(valid, inv, 1.0)
        std = np.where(valid, std, 1.0)

        scal2_host[k, 0, :] = (-mu).astype(bf16)
        scal2_host[k, 1, :] = std.astype(bf16)
        std1_host[k, 0, :] = std.astype(bf16)
        invc_host[k] = inv.astype(np.float32).reshape(ntp, P).T

        ic = 0
        for (t, cl, tile0, ntile) in segments:
            ne = ntile * P
            base = tile0 * P
            rr = r[base : base + ne].copy()
            cc = c[base : base + ne].copy()
            rr = rr - (NLO if cl in (2, 3) else 0)
            cc = cc - (NLO if cl in (1, 3) else 0)
            rr = np.maximum(rr, 0)
            cc = np.maximum(cc, 0)
            for off, ni in _chunks(ne, GCHUNK):
                idx_host[k, :, ic : ic + ni // 16] = _wrap_idx(rr[off : off + ni], ni)
                ic += ni // 16
            for off, ni in _chunks(ne, GCHUNK):
                idx_host[k, :, ic : ic + ni // 16] = _wrap_idx(cc[off : off + ni], ni)
                ic += ni // 16
        assert ic == idxcols

    x_bf = np.ascontiguousarray(x.astype(bf16))
    w1a_host = np.ascontiguousarray(
        W1e[:, :C, :].astype(bf16).transpose(1, 0, 2).reshape(P, T * H)
    )
    w1b_host = np.ascontiguousarray(
        W1e[:, C:, :].astype(bf16).transpose(1, 0, 2).reshape(P, T * H)
    )
    wc_host = np.ascontiguousarray(
        np.stack([u, b1e], axis=1).astype(bf16).transpose(1, 0, 2).reshape(2, T * H)
    )
    w2aug = np.concatenate([W2, b2[:, None, :]], axis=1)  # [T, H+1, DD]
    w2_host = np.ascontiguousarray(
        w2aug.astype(bf16).transpose(1, 0, 2).reshape(H + 1, T * DD)
    )
    mbt = ntp // SMCH
    eye_host = np.ascontiguousarray(
        np.broadcast_to(
            np.tile(np.eye(D, dtype=np.float32).reshape(DD), mbt), (P, mbt * DD)
        )
    )
    return dict(
        xbf=x_bf, idx=idx_host, scal2=scal2_host, std1=std1_host, invc=invc_host,
        w1a=w1a_host, w1b=w1b_host, wc=wc_host, w2aug=w2_host, eye=eye_host,
        eids=eids, segments=tuple(segments), ntp=ntp, idxcols=idxcols,
    )


_LAST_RESULTS = {}


def kernel(x, edge_index, edge_types, gamma, beta, W1, b1, W2, b2):
    from concourse.bass_utils import run_bass_kernel_spmd

    prep = _prepare(x, edge_index, edge_types, gamma, beta, W1, b1, W2, b2)
    segments, ntp, idxcols = prep["segments"], prep["ntp"], prep["idxcols"]

    key = (segments, ntp, idxcols)
    nc = _PROGRAM_CACHE.get(key)
    if nc is None:
        nc = _build_program(segments, ntp, idxcols)
        _PROGRAM_CACHE[key] = nc

    in_maps = [
        dict(
            xbf=prep["xbf"], idxw=prep["idx"][k], scal2=prep["scal2"][k],
            std1=prep["std1"][k], invc=prep["invc"][k], w1a=prep["w1a"],
            w1b=prep["w1b"], wc=prep["wc"], w2aug=prep["w2aug"], eyeb=prep["eye"],
        )
        for k in range(NCORES)
    ]
    trace = bool(int(os.environ.get("KERNEL_TRACE", "0")))
    res = run_bass_kernel_spmd(
        nc, in_maps, core_ids=list(range(NCORES)), trace=trace
    )
    _LAST_RESULTS["res"] = res

    out = np.zeros((E, DD), dtype=np.float32)
    for k in range(NCORES):
        o = (
            np.asarray(res.results[k]["out"])
            .reshape(P, ntp, DD)
            .transpose(1, 0, 2)
            .reshape(-1, DD)
        )
        e = prep["eids"][k]
        valid = e >= 0
        out[e[valid]] = o[valid]
    return out.reshape(E, D, D)
